# revision 8
# baseline (speedup 1.0000x reference)
"""Trainium2 Bass kernel for nn_ModelGCNAttn3h (GCN + attention + ROI pooling).

Self-contained: hardcodes shapes/sharding. kernel(**inputs) takes full inputs,
shards across 8 NeuronCores (2 graphs per core), runs one SPMD NEFF, gathers.
"""
import math

import numpy as np

import concourse.bacc as bacc
import concourse.bass as bass
import concourse.mybir as mybir
import concourse.tile as tile
from concourse import bass_utils
from concourse.masks import make_identity

FP = mybir.dt.float32
I32 = mybir.dt.int32

B = 16
NPER = 1024
N = B * NPER
NROI = 148
N2 = B * NROI
DIN = 64
H = 128
NHEADS = 4
HD = 32
OUT_D = 2
EPS = 1e-5
NCORE = 8
GB = B // NCORE            # graphs per core = 2
NLV = GB * NPER            # 2048 vertex nodes per core
NLR = GB * NROI            # 296 roi nodes per core
NWV = NLV // 128           # 16 vertex windows
NWR = math.ceil(NLR / 128)  # 3 roi windows
ROWS_PER_CORE = NLV + NLR  # 2344 rows contributed to gather table
TBL_ROWS = NCORE * ROWS_PER_CORE
G = 4                      # blocks fused per supergather
BLK = 128
ISCALE = 1.0 / math.sqrt(HD)

_CACHE = {}


# ---------------------------------------------------------------- host prep

def _shard_edges(src, dst, w, n_nodes, npc, nwin, srow_of):
    """Sort edges (incl self loops) by (core, window); pad per-window block
    counts to a global (multiple-of-G) max. Returns per-core arrays shaped
    (NSB, 128, G) for gather row idx / weight / local dst, plus degw layout."""
    src = np.concatenate([src.astype(np.int64), np.arange(n_nodes, dtype=np.int64)])
    dst = np.concatenate([dst.astype(np.int64), np.arange(n_nodes, dtype=np.int64)])
    w = np.concatenate([np.asarray(w, np.float32), np.ones(n_nodes, np.float32)])
    core = dst // npc
    dloc = dst - core * npc
    win = dloc // BLK
    ldst = (dloc % BLK).astype(np.float32)
    key = core * nwin + win
    cnt = np.bincount(key, minlength=NCORE * nwin).reshape(NCORE, nwin)
    nbw = (np.maximum(np.ceil(cnt.max(0) / (BLK * G)), 1) * G).astype(int)  # blocks/window
    nblk = int(nbw.sum())
    assert nblk % G == 0
    srow = srow_of(src)

    order = np.argsort(key, kind="stable")
    starts = np.zeros(NCORE * nwin + 1, np.int64)
    np.cumsum(cnt.reshape(-1), out=starts[1:])

    e_idx = np.zeros((NCORE, nblk * BLK), np.int32)
    e_w = np.zeros((NCORE, nblk * BLK), np.float32)
    e_ld = np.full((NCORE, nblk * BLK), -1.0, np.float32)
    woff = np.zeros(nwin + 1, np.int64)
    np.cumsum(nbw * BLK, out=woff[1:])
    for c in range(NCORE):
        for wi in range(nwin):
            s, e = starts[c * nwin + wi], starts[c * nwin + wi + 1]
            sel = order[s:e]
            o = woff[wi]
            e_idx[c, o:o + len(sel)] = srow[sel]
            e_w[c, o:o + len(sel)] = w[sel]
            e_ld[c, o:o + len(sel)] = ldst[sel]

    def to_sb(a):  # (nblk*128,) -> (NSB, 128, G)
        return np.ascontiguousarray(
            a.reshape(nblk // G, G, BLK).transpose(0, 2, 1))

    e_idx = np.stack([to_sb(e_idx[c]) for c in range(NCORE)])
    e_w = np.stack([to_sb(e_w[c]) for c in range(NCORE)])
    e_ld = np.stack([to_sb(e_ld[c]) for c in range(NCORE)])

    # degw: per-core (nwin*128, DEGCAP) in-edge weights (incl self), pads deg=1
    dcap = int(np.bincount(dst, minlength=n_nodes).max())
    dcap = ((dcap + 3) // 4) * 4
    npad = nwin * BLK
    degw = np.zeros((NCORE, npad, dcap), np.float32)
    ordd = np.argsort(dst, kind="stable")
    dcnt = np.bincount(dst, minlength=n_nodes)
    dstart = np.zeros(n_nodes + 1, np.int64)
    np.cumsum(dcnt, out=dstart[1:])
    pos = np.arange(len(dst)) - dstart[dst[ordd]]
    crow = dst[ordd] // npc
    lrow = dst[ordd] - crow * npc
    degw[crow, lrow, pos] = w[ordd]
    if npc < npad:
        degw[:, npc:, 0] = 1.0
    return e_idx, e_w, e_ld, [int(x) for x in nbw], degw, dcap


def _prep(inputs):
    x = np.asarray(inputs["x"], np.float32)
    node_roi = np.asarray(inputs["node_roi"], np.int64)
    batch = np.asarray(inputs["batch"], np.int64)
    ei = np.asarray(inputs["edge_index"], np.int64)
    ew = np.asarray(inputs["edge_weight"], np.float32)
    x2 = np.asarray(inputs["x2"], np.float32)
    roi2 = np.asarray(inputs["roi_label2"], np.int64)
    batch2 = np.asarray(inputs["batch2"], np.int64)
    ei2 = np.asarray(inputs["edge_index2"], np.int64)
    ew2 = np.asarray(inputs["edge_weight2"], np.float32)
    assert np.array_equal(batch, np.repeat(np.arange(B), NPER)), "non-uniform batch"
    assert np.array_equal(batch2, np.repeat(np.arange(B), NROI)), "non-uniform batch2"
    assert np.array_equal(roi2, np.tile(np.arange(NROI), B)), "roi_label2 not arange"

    def srow_v(s):
        return (s + NLR * (s // NLV)).astype(np.int32)

    def srow_r(s):
        return (NLV * (s // NLR) + NLV + s).astype(np.int32)

    eiv, ewv, eldv, nbw_v, degw_v, dcap_v = _shard_edges(
        ei[0], ei[1], ew, N, NLV, NWV, srow_v)
    eir, ewr, eldr, nbw_r, degw_r, dcap_r = _shard_edges(
        ei2[0], ei2[1], ew2, N2, NLR, NWR, srow_r)

    # pooling: per graph, node local-row order sorted by roi + counts
    pool_idx = np.zeros((NCORE, GB * (NPER // (BLK * G)), BLK, G), np.int32)
    pool_roi = np.zeros((NCORE, GB * (NPER // (BLK * G)), BLK, G), np.float32)
    pool_cnt = np.zeros((NCORE, GB, NROI), np.float32)
    nsb_g = NPER // (BLK * G)  # 2 supergathers per graph
    for g in range(B):
        c, gl = g // GB, g % GB
        roi = node_roi[g * NPER:(g + 1) * NPER]
        ordg = np.argsort(roi, kind="stable")
        flat_i = (gl * NPER + ordg).astype(np.int32)
        flat_r = roi[ordg].astype(np.float32)
        pool_idx[c, gl * nsb_g:(gl + 1) * nsb_g] = (
            flat_i.reshape(nsb_g, G, BLK).transpose(0, 2, 1))
        pool_roi[c, gl * nsb_g:(gl + 1) * nsb_g] = (
            flat_r.reshape(nsb_g, G, BLK).transpose(0, 2, 1))
        pool_cnt[c, gl] = np.bincount(roi, minlength=NROI).astype(np.float32)

    # params
    p3 = {k: np.asarray(v, np.float32) for k, v in inputs["params_mha3h"].items()}
    p2 = {k: np.asarray(v, np.float32) for k, v in inputs["params_attn_sum"].items()}
    pm = {k: np.asarray(v, np.float32) for k, v in inputs["params_mlp"].items()}
    gv = [(np.asarray(W, np.float32), np.asarray(b, np.float32))
          for W, b in inputs["params_gcn"]]
    gr = [(np.asarray(W, np.float32), np.asarray(b, np.float32))
          for W, b in inputs["params_gcn_roi"]]

    rep = lambda v: np.ascontiguousarray(np.tile(v.reshape(1, -1), (128, 1)))
    col = lambda v: np.ascontiguousarray(v.reshape(-1, 1))
    shared = {
        "W1_v": gv[0][0], "b1_v_rep": rep(gv[0][1]),
        "W2_v": gv[1][0], "b2_v_rep": rep(gv[1][1]),
        "W1_r": gr[0][0], "b1_r_rep": rep(gr[0][1]),
        "W2_r": gr[1][0], "b2_r_rep": rep(gr[1][1]),
        "Wq3T": np.ascontiguousarray(p3["in_w"][:H].T),
        "Wk3T": np.ascontiguousarray(p3["in_w"][H:2 * H].T),
        "bq3": col(p3["in_b"][:H]), "bk3": col(p3["in_b"][H:2 * H]),
        "WqT2": np.ascontiguousarray(p2["in_w"][:H].T),
        "WkT2": np.ascontiguousarray(p2["in_w"][H:2 * H].T),
        "WvT2": np.ascontiguousarray(p2["in_w"][2 * H:].T),
        "bq2": col(p2["in_b"][:H]), "bk2": col(p2["in_b"][H:2 * H]),
        "bv2_rep": rep(p2["in_b"][2 * H:]),
        "outwT2": np.ascontiguousarray(p2["out_w"].T),
        "outb2_rep": rep(p2["out_b"]),
        "ln1g_rep": rep(p2["ln1_g"]), "ln1b_rep": rep(p2["ln1_b"]),
        "ff1wT": np.ascontiguousarray(p2["ff1_w"].T), "ff1b_rep": rep(p2["ff1_b"]),
        "ff2wT": np.ascontiguousarray(p2["ff2_w"].T), "ff2b_rep": rep(p2["ff2_b"]),
        "ln2g_rep": rep(p2["ln2_g"]), "ln2b_rep": rep(p2["ln2_b"]),
        "b2_out_rep": np.ascontiguousarray(np.tile(pm["b2"].reshape(1, -1), (B, 1))),
    }

    # MLP per-core: 125 output neurons each
    NSL = 1000 // NCORE
    bn_scale = pm["bn_g"] / math.sqrt(1.0 + EPS)
    bn_bias = pm["b1"] * bn_scale + pm["bn_b"]
    in_maps = []
    for c in range(NCORE):
        w1s = pm["w1"][c * NSL:(c + 1) * NSL]          # (125, 18944)
        w1sT = np.zeros((NROI * H, 128), np.float32)
        w1sT[:, :NSL] = w1s.T
        w2sT = np.zeros((128, OUT_D), np.float32)
        w2sT[:NSL] = pm["w2"][:, c * NSL:(c + 1) * NSL].T
        msc = np.zeros((128, 1), np.float32)
        msc[:NSL, 0] = bn_scale[c * NSL:(c + 1) * NSL]
        mbi = np.zeros((128, 1), np.float32)
        mbi[:NSL, 0] = bn_bias[c * NSL:(c + 1) * NSL]
        m = {
            "x_loc": np.ascontiguousarray(x[c * NLV:(c + 1) * NLV]),
            "x2_loc": np.ascontiguousarray(
                np.vstack([x2[c * NLR:(c + 1) * NLR],
                           np.zeros((NWR * BLK - NLR, DIN), np.float32)])),
            "degw_v": degw_v[c], "degw_r": degw_r[c],
            "e_idx_v": eiv[c], "e_w_v": ewv[c], "e_ldst_v": eldv[c],
            "e_idx_r": eir[c], "e_w_r": ewr[c], "e_ldst_r": eldr[c],
            "pool_idx": pool_idx[c], "pool_roi": pool_roi[c],
            "pool_cnt": pool_cnt[c],
            "w1sT": w1sT, "w2sT": w2sT, "mlp_scale": msc, "mlp_bias": mbi,
        }
        m.update(shared)
        in_maps.append(m)

    cfg = (tuple(nbw_v), tuple(nbw_r), dcap_v, dcap_r)
    return in_maps, cfg


# ---------------------------------------------------------------- device build

def _build(cfg):
    nbw_v, nbw_r, dcap_v, dcap_r = cfg
    nsb_v = sum(nbw_v) // G
    nsb_r = sum(nbw_r) // G
    nsb_p = GB * (NPER // (BLK * G))

    nc = bacc.Bacc("TRN2", target_bir_lowering=False, debug=False,
                   num_devices=NCORE)

    def di(name, shape, dt=FP):
        return nc.dram_tensor(name, list(shape), dt, kind="ExternalInput").ap()

    def do(name, shape, dt=FP):
        return nc.dram_tensor(name, list(shape), dt, kind="ExternalOutput").ap()

    x_loc = di("x_loc", (NLV, DIN))
    x2_loc = di("x2_loc", (NWR * BLK, DIN))
    degw_v = di("degw_v", (NWV * BLK, dcap_v))
    degw_r = di("degw_r", (NWR * BLK, dcap_r))
    e_idx_v = di("e_idx_v", (nsb_v, BLK, G), I32)
    e_w_v = di("e_w_v", (nsb_v, BLK, G))
    e_ld_v = di("e_ldst_v", (nsb_v, BLK, G))
    e_idx_r = di("e_idx_r", (nsb_r, BLK, G), I32)
    e_w_r = di("e_w_r", (nsb_r, BLK, G))
    e_ld_r = di("e_ldst_r", (nsb_r, BLK, G))
    pool_idx = di("pool_idx", (nsb_p, BLK, G), I32)
    pool_roi = di("pool_roi", (nsb_p, BLK, G))
    pool_cnt = di("pool_cnt", (GB, NROI))
    w1sT = di("w1sT", (NROI * H, 128))
    w2sT = di("w2sT", (128, OUT_D))
    mlp_scale = di("mlp_scale", (128, 1))
    mlp_bias = di("mlp_bias", (128, 1))

    W1_v = di("W1_v", (DIN, H)); b1_v = di("b1_v_rep", (128, H))
    W2_v = di("W2_v", (H, H)); b2_v = di("b2_v_rep", (128, H))
    W1_r = di("W1_r", (DIN, H)); b1_r = di("b1_r_rep", (128, H))
    W2_r = di("W2_r", (H, H)); b2_r = di("b2_r_rep", (128, H))
    Wq3T = di("Wq3T", (H, H)); Wk3T = di("Wk3T", (H, H))
    bq3 = di("bq3", (H, 1)); bk3 = di("bk3", (H, 1))
    WqT2 = di("WqT2", (H, H)); WkT2 = di("WkT2", (H, H)); WvT2 = di("WvT2", (H, H))
    bq2 = di("bq2", (H, 1)); bk2 = di("bk2", (H, 1)); bv2 = di("bv2_rep", (128, H))
    outwT2 = di("outwT2", (H, H)); outb2 = di("outb2_rep", (128, H))
    ln1g = di("ln1g_rep", (128, H)); ln1b = di("ln1b_rep", (128, H))
    ff1wT = di("ff1wT", (H, H)); ff1b = di("ff1b_rep", (128, H))
    ff2wT = di("ff2wT", (H, H)); ff2b = di("ff2b_rep", (128, H))
    ln2g = di("ln2g_rep", (128, H)); ln2b = di("ln2b_rep", (128, H))
    b2_out = di("b2_out_rep", (B, OUT_D))

    o_logits = do("o_logits", (B, OUT_D))
    o_emb = do("o_emb", (GB, NROI, H))
    o_embroi = do("o_embroi", (GB, NROI, H))
    o_comb = do("o_comb", (GB, NROI, H))
    o_tout = do("o_tout", (GB, NROI, H))
    o_attnw = do("o_attnw", (GB, NHEADS, NROI, NROI))
    o_emb3h = do("o_emb3h", (NLV, H))
    o_w3h = do("o_w3h", (GB, NHEADS, NPER, NPER))

    RG = [list(range(NCORE))]

    with tile.TileContext(nc) as tc:
        with (
            tc.tile_pool(name="persist", bufs=1) as pp,
            tc.tile_pool(name="work", bufs=3) as wp,
            tc.tile_pool(name="psum", bufs=2, space="PSUM") as psp,
            tc.tile_pool(name="dram", bufs=1, space="DRAM") as dp,
        ):
            ident = pp.tile([128, 128], FP, name="ident")
            make_identity(nc, ident[:])
            iota128 = pp.tile([128, G * 128], FP, name="iota128")
            iota128_i = pp.tile([128, G * 128], I32, name="iota128_i")
            nc.gpsimd.iota(iota128_i[:].rearrange("p (g w) -> p g w", g=G),
                           pattern=[[0, G], [1, 128]], base=0, channel_multiplier=0)
            nc.vector.tensor_copy(iota128[:], iota128_i[:])
            iota148 = pp.tile([128, G * NROI], FP, name="iota148")
            iota148_i = pp.tile([128, G * NROI], I32, name="iota148_i")
            eps_t = pp.tile([128, 1], FP, name="eps_t")
            nc.gpsimd.memset(eps_t[:], EPS)
            nc.gpsimd.iota(iota148_i[:].rearrange("p (g w) -> p g w", g=G),
                           pattern=[[0, G], [1, NROI]], base=0, channel_multiplier=0)
            nc.vector.tensor_copy(iota148[:], iota148_i[:])

            # ---------------- stage 1: deg -> dinv -> x' ; AG1
            ag1_in = dp.tile([ROWS_PER_CORE, DIN], FP, name="ag1_in")
            ag1_out = dp.tile([TBL_ROWS, DIN], FP, name="ag1_out",
                              addr_space="Shared")

            def deg_stage(degw_d, nwin, dcap, nm):
                dw = wp.tile([128, nwin, dcap], FP, name=f"dw_{nm}", bufs=1)
                nc.sync.dma_start(
                    dw[:], degw_d.rearrange("(w p) c -> p w c", p=128))
                deg = pp.tile([128, nwin], FP, name=f"deg_{nm}")
                nc.vector.reduce_sum(deg[:].unsqueeze(-1), dw[:],
                                     axis=mybir.AxisListType.X)
                nc.scalar.sqrt(deg[:], deg[:])
                dinv = pp.tile([128, nwin], FP, name=f"dinv_{nm}")
                nc.vector.reciprocal(dinv[:], deg[:])
                return dinv

            dinv_v = deg_stage(degw_v, NWV, dcap_v, "v")
            dinv_r = deg_stage(degw_r, NWR, dcap_r, "r")

            xv = wp.tile([128, NWV, DIN], FP, name="xv", bufs=1)
            nc.sync.dma_start(xv[:], x_loc.rearrange("(w p) d -> p w d", p=128))
            nc.vector.tensor_tensor(
                out=xv[:], in0=xv[:],
                in1=dinv_v[:].unsqueeze(-1).broadcast_to([128, NWV, DIN]),
                op=mybir.AluOpType.mult)
            nc.sync.dma_start(
                ag1_in[:NLV].rearrange("(w p) d -> p w d", p=128), xv[:])

            xr = wp.tile([128, NWR, DIN], FP, name="xr", bufs=1)
            nc.sync.dma_start(xr[:], x2_loc.rearrange("(w p) d -> p w d", p=128))
            nc.vector.tensor_tensor(
                out=xr[:], in0=xr[:],
                in1=dinv_r[:].unsqueeze(-1).broadcast_to([128, NWR, DIN]),
                op=mybir.AluOpType.mult)
            for wi in range(NWR):
                lo = NLV + wi * BLK
                sz = min(BLK, NLR - wi * BLK)
                nc.sync.dma_start(ag1_in[lo:lo + sz], xr[:sz, wi])
            nc.gpsimd.collective_compute(
                "AllGather", mybir.AluOpType.bypass, replica_groups=RG,
                ins=[ag1_in.opt()], outs=[ag1_out.opt()])

            # ---------------- GCN sweep helper
            def load_edges(e_idx_d, e_w_d, e_ld_d, nsb, nm):
                it = pp.tile([128, nsb, G], I32, name=f"eidx_{nm}")
                wt = pp.tile([128, nsb, G], FP, name=f"ew_{nm}")
                lt = pp.tile([128, nsb, G], FP, name=f"eld_{nm}")
                nc.sync.dma_start(it[:], e_idx_d.rearrange("s p g -> p s g"))
                nc.sync.dma_start(wt[:], e_w_d.rearrange("s p g -> p s g"))
                nc.sync.dma_start(lt[:], e_ld_d.rearrange("s p g -> p s g"))
                return it, wt, lt

            ev = load_edges(e_idx_v, e_w_v, e_ld_v, nsb_v, "v")
            er = load_edges(e_idx_r, e_w_r, e_ld_r, nsb_r, "r")

            def gcn_sweep(nm, table, din, nbw, edges, dinv, Wt, brt, out_cb):
                """One GCN layer over this core's windows. out_cb(wi, sz, pre_ap)
                consumes the pre-relu psum-h (128, H) sbuf tile per window."""
                it, wt, lt = edges
                sb0 = 0
                for wi, nb in enumerate(nbw):
                    agg = psp.tile([128, din], FP, name=f"agg_{nm}", tag="acc", space="PSUM")
                    for k in range(nb // G):
                        sb = sb0 + k
                        gt = wp.tile([128, G * din], FP, name=f"gth_{nm}")
                        for gg in range(G):
                            nc.gpsimd.indirect_dma_start(
                                out=gt[:, gg * din:(gg + 1) * din],
                                out_offset=None, in_=table[:],
                                in_offset=bass.IndirectOffsetOnAxis(
                                    ap=it[:, sb, gg:gg + 1], axis=0))
                        oh = wp.tile([128, G * 128], FP, name=f"oh_{nm}")
                        nc.vector.tensor_tensor(
                            out=oh[:].rearrange("p (g w) -> p g w", g=G),
                            in0=lt[:, sb].unsqueeze(-1).broadcast_to([128, G, 128]),
                            in1=iota128[:].rearrange("p (g w) -> p g w", g=G),
                            op=mybir.AluOpType.is_equal)
                        nc.vector.tensor_tensor(
                            out=gt[:].rearrange("p (g d) -> p g d", g=G),
                            in0=gt[:].rearrange("p (g d) -> p g d", g=G),
                            in1=wt[:, sb].unsqueeze(-1).broadcast_to([128, G, din]),
                            op=mybir.AluOpType.mult)
                        for g in range(G):
                            nc.tensor.matmul(
                                agg[:], lhsT=oh[:, g * 128:(g + 1) * 128],
                                rhs=gt[:, g * din:(g + 1) * din],
                                start=(k == 0 and g == 0),
                                stop=(k == nb // G - 1 and g == G - 1))
                    sb0 += nb // G
                    aggs = wp.tile([128, din], FP, name=f"aggs_{nm}")
                    nc.scalar.activation(aggs[:], agg[:],
                                         mybir.ActivationFunctionType.Copy,
                                         bias=0.0, scale=dinv[:, wi:wi + 1])
                    tp = psp.tile([din, 128], FP, name=f"tp_{nm}", tag="tr", space="PSUM")
                    nc.tensor.transpose(tp[:], aggs[:], ident[:])
                    aggT = wp.tile([din, 128], FP, name=f"aggT_{nm}")
                    nc.vector.tensor_copy(aggT[:], tp[:])
                    hp = psp.tile([128, H], FP, name=f"hp_{nm}", tag="acc2", space="PSUM")
                    nc.tensor.matmul(hp[:], lhsT=aggT[:], rhs=Wt[:],
                                     start=True, stop=True)
                    pre = wp.tile([128, H], FP, name=f"pre_{nm}")
                    nc.vector.tensor_tensor(out=pre[:], in0=hp[:], in1=brt[:],
                                            op=mybir.AluOpType.add)
                    out_cb(wi, pre)

            # weights to SBUF
            def wtile(d, shape, nm):
                t = pp.tile(list(shape), FP, name=nm)
                nc.sync.dma_start(t[:], d[:])
                return t

            W1v_t = wtile(W1_v, (DIN, H), "W1v_t")
            b1v_t = wtile(b1_v, (128, H), "b1v_t")
            W2v_t = wtile(W2_v, (H, H), "W2v_t")
            b2v_t = wtile(b2_v, (128, H), "b2v_t")
            W1r_t = wtile(W1_r, (DIN, H), "W1r_t")
            b1r_t = wtile(b1_r, (128, H), "b1r_t")
            W2r_t = wtile(W2_r, (H, H), "W2r_t")
            b2r_t = wtile(b2_r, (128, H), "b2r_t")

            # ---------------- stage 2: layer 1 (vertex + roi) -> AG2
            ag2_in = dp.tile([ROWS_PER_CORE, H], FP, name="ag2_in")
            ag2_out = dp.tile([TBL_ROWS, H], FP, name="ag2_out",
                              addr_space="Shared")

            def l1v_cb(wi, pre):
                h1p = wp.tile([128, H], FP, name="h1p_v")
                nc.scalar.activation(h1p[:], pre[:],
                                     mybir.ActivationFunctionType.Relu,
                                     bias=0.0, scale=dinv_v[:, wi:wi + 1])
                nc.sync.dma_start(ag2_in[wi * BLK:(wi + 1) * BLK], h1p[:])

            gcn_sweep("v1", ag1_out, DIN, nbw_v, ev, dinv_v, W1v_t, b1v_t, l1v_cb)

            def l1r_cb(wi, pre):
                h1p = wp.tile([128, H], FP, name="h1p_r")
                nc.scalar.activation(h1p[:], pre[:],
                                     mybir.ActivationFunctionType.Relu,
                                     bias=0.0, scale=dinv_r[:, wi:wi + 1])
                lo = NLV + wi * BLK
                sz = min(BLK, NLR - wi * BLK)
                nc.sync.dma_start(ag2_in[lo:lo + sz], h1p[:sz])

            gcn_sweep("r1", ag1_out, DIN, nbw_r, er, dinv_r, W1r_t, b1r_t, l1r_cb)

            nc.gpsimd.collective_compute(
                "AllGather", mybir.AluOpType.bypass, replica_groups=RG,
                ins=[ag2_in.opt()], outs=[ag2_out.opt()])

            # ---------------- stage 3: layer 2 -> h2 (vertex), h2r (roi)
            h2 = pp.tile([128, NWV, H], FP, name="h2")       # window-major
            h2_dram = dp.tile([NLV, H], FP, name="h2_dram")

            def l2v_cb(wi, pre):
                nc.scalar.activation(h2[:, wi], pre[:],
                                     mybir.ActivationFunctionType.Relu)
                nc.sync.dma_start(o_emb3h[wi * BLK:(wi + 1) * BLK], h2[:, wi])
                nc.sync.dma_start(h2_dram[wi * BLK:(wi + 1) * BLK], h2[:, wi])

            gcn_sweep("v2", ag2_out, H, nbw_v, ev, dinv_v, W2v_t, b2v_t, l2v_cb)

            embroi_dram = dp.tile([NLR, H], FP, name="embroi_dram")

            def l2r_cb(wi, pre):
                h2r = wp.tile([128, H], FP, name="h2r")
                nc.scalar.activation(h2r[:], pre[:],
                                     mybir.ActivationFunctionType.Relu)
                lo = wi * BLK
                sz = min(BLK, NLR - lo)
                nc.sync.dma_start(embroi_dram[lo:lo + sz], h2r[:sz])

            gcn_sweep("r2", ag2_out, H, nbw_r, er, dinv_r, W2r_t, b2r_t, l2r_cb)
            nc.sync.dma_start(o_embroi[:].rearrange("g r h -> (g r) h"),
                              embroi_dram[:])

            # ---------------- stage 4: ROI-mean pooling + combined
            pidx_t = pp.tile([128, nsb_p, G], I32, name="pidx_t")
            proi_t = pp.tile([128, nsb_p, G], FP, name="proi_t")
            nc.sync.dma_start(pidx_t[:], pool_idx.rearrange("s p g -> p s g"))
            nc.sync.dma_start(proi_t[:], pool_roi.rearrange("s p g -> p s g"))
            cnt_t = pp.tile([128, GB, 2], FP, name="cnt_t")  # [p,(g,chunk)]
            nc.gpsimd.memset(cnt_t[:], 1.0)
            for g in range(GB):
                nc.sync.dma_start(cnt_t[:, g, 0].unsqueeze(-1),
                                  pool_cnt[g, :128].unsqueeze(-1))
                nc.sync.dma_start(cnt_t[:NROI - 128, g, 1].unsqueeze(-1),
                                  pool_cnt[g, 128:].unsqueeze(-1))
            nc.vector.tensor_scalar_max(cnt_t[:], cnt_t[:], 1.0)
            rcnt_t = pp.tile([128, GB, 2], FP, name="rcnt_t")
            nc.vector.reciprocal(rcnt_t[:], cnt_t[:])

            cmb = pp.tile([128, GB, 2, H], FP, name="cmb")  # [p,(g,chunk),H]
            nsg = NPER // (BLK * G)
            for g in range(GB):
                psA = psp.tile([128, H], FP, name="psA", tag="acc", space="PSUM")
                psB = psp.tile([NROI - 128, H], FP, name="psB", tag="acc2", space="PSUM")
                for k in range(nsg):
                    sb = g * nsg + k
                    gt = wp.tile([128, G * H], FP, name="gth_p")
                    for gg in range(G):
                        nc.gpsimd.indirect_dma_start(
                            out=gt[:, gg * H:(gg + 1) * H],
                            out_offset=None, in_=h2_dram[:],
                            in_offset=bass.IndirectOffsetOnAxis(
                                ap=pidx_t[:, sb, gg:gg + 1], axis=0))
                    oh = wp.tile([128, G * NROI], FP, name="oh_p")
                    nc.vector.tensor_tensor(
                        out=oh[:].rearrange("p (g w) -> p g w", g=G),
                        in0=proi_t[:, sb].unsqueeze(-1).broadcast_to(
                            [128, G, NROI]),
                        in1=iota148[:].rearrange("p (g w) -> p g w", g=G),
                        op=mybir.AluOpType.is_equal)
                    for gg in range(G):
                        nc.tensor.matmul(
                            psA[:], lhsT=oh[:, gg * NROI:gg * NROI + 128],
                            rhs=gt[:, gg * H:(gg + 1) * H],
                            start=(k == 0 and gg == 0),
                            stop=(k == nsg - 1 and gg == G - 1),
                            skip_group_check=True)
                    for gg in range(G):
                        nc.tensor.matmul(
                            psB[:], lhsT=oh[:, gg * NROI + 128:(gg + 1) * NROI],
                            rhs=gt[:, gg * H:(gg + 1) * H],
                            start=(k == 0 and gg == 0),
                            stop=(k == nsg - 1 and gg == G - 1),
                            skip_group_check=True)
                embA = wp.tile([128, H], FP, name="embA", bufs=2)
                nc.scalar.activation(embA[:], psA[:],
                                     mybir.ActivationFunctionType.Copy,
                                     bias=0.0, scale=rcnt_t[:, g, 0].unsqueeze(-1))
                embB = wp.tile([NROI - 128, H], FP, name="embB", bufs=2)
                nc.scalar.activation(embB[:], psB[:],
                                     mybir.ActivationFunctionType.Copy,
                                     bias=0.0,
                                     scale=rcnt_t[:NROI - 128, g, 1].unsqueeze(-1))
                nc.sync.dma_start(o_emb[g, :128], embA[:])
                nc.sync.dma_start(o_emb[g, 128:], embB[:])
                # combined = emb + embroi
                roiA = wp.tile([128, H], FP, name="roiA", bufs=2)
                nc.sync.dma_start(roiA[:], embroi_dram[g * NROI:g * NROI + 128])
                roiB = wp.tile([NROI - 128, H], FP, name="roiB", bufs=2)
                nc.sync.dma_start(roiB[:], embroi_dram[g * NROI + 128:(g + 1) * NROI])
                nc.vector.tensor_tensor(out=cmb[:, g, 0], in0=embA[:], in1=roiA[:],
                                        op=mybir.AluOpType.add)
                nc.vector.tensor_tensor(out=cmb[:NROI - 128, g, 1], in0=embB[:],
                                        in1=roiB[:], op=mybir.AluOpType.add)
                nc.sync.dma_start(o_comb[g, :128], cmb[:, g, 0])
                nc.sync.dma_start(o_comb[g, 128:], cmb[:NROI - 128, g, 1])

            # ---------------- stage 5: 3h attention weights (w3h)
            Wq3_t = wtile(Wq3T, (H, H), "Wq3_t")
            Wk3_t = wtile(Wk3T, (H, H), "Wk3_t")
            bq3_t = wtile(bq3, (H, 1), "bq3_t")
            bk3_t = wtile(bk3, (H, 1), "bk3_t")
            for g in range(GB):
                h2T = wp.tile([128, NPER], FP, name="h2T", bufs=2)
                for wi in range(8):
                    tp = psp.tile([128, 128], FP, name="tp_a3", tag="tr", space="PSUM")
                    nc.tensor.transpose(tp[:], h2[:, g * 8 + wi], ident[:])
                    nc.vector.tensor_copy(h2T[:, wi * 128:(wi + 1) * 128], tp[:])
                qT = wp.tile([128, NPER], FP, name="qT", bufs=2)
                kT = wp.tile([128, NPER], FP, name="kT", bufs=2)
                for half in range(2):
                    s = slice(half * 512, (half + 1) * 512)
                    pq = psp.tile([128, 512], FP, name="pq", tag="acc2", space="PSUM")
                    nc.tensor.matmul(pq[:], lhsT=Wq3_t[:], rhs=h2T[:, s],
                                     start=True, stop=True)
                    nc.scalar.activation(qT[:, s], pq[:],
                                         mybir.ActivationFunctionType.Identity,
                                         bias=bq3_t[:, :1])
                    pk = psp.tile([128, 512], FP, name="pk", tag="acc2", space="PSUM")
                    nc.tensor.matmul(pk[:], lhsT=Wk3_t[:], rhs=h2T[:, s],
                                     start=True, stop=True)
                    nc.scalar.activation(kT[:, s], pk[:],
                                         mybir.ActivationFunctionType.Identity,
                                         bias=bk3_t[:, :1])
                for h in range(NHEADS):
                    hs = slice(h * HD, (h + 1) * HD)
                    for qi in range(NPER // 128):
                        ps = psp.tile([128, NPER], FP, name="ps_sc", tag="acc", space="PSUM")
                        for half in range(2):
                            nc.tensor.matmul(
                                ps[:, half * 512:(half + 1) * 512],
                                lhsT=qT[hs, qi * 128:(qi + 1) * 128],
                                rhs=kT[hs, half * 512:(half + 1) * 512],
                                start=True, stop=True,
                                tile_position=(h * HD, 0))
                        et = wp.tile([128, NPER], FP, name="et")
                        rs = wp.tile([128, 1], FP, name="rs")
                        nc.scalar.activation(et[:], ps[:],
                                             mybir.ActivationFunctionType.Exp,
                                             bias=0.0, scale=ISCALE,
                                             accum_out=rs[:])
                        nc.vector.reciprocal(rs[:], rs[:])
                        nc.vector.tensor_scalar_mul(et[:], et[:], rs[:, :1])
                        nc.sync.dma_start(
                            o_w3h[g, h, qi * 128:(qi + 1) * 128], et[:])

            # ---------------- stage 6: summary attention block (attn2)
            WqT2_t = wtile(WqT2, (H, H), "WqT2_t")
            WkT2_t = wtile(WkT2, (H, H), "WkT2_t")
            WvT2_t = wtile(WvT2, (H, H), "WvT2_t")
            bq2_t = wtile(bq2, (H, 1), "bq2_t")
            bk2_t = wtile(bk2, (H, 1), "bk2_t")
            bv2_t = wtile(bv2, (128, H), "bv2_t")
            outwT2_t = wtile(outwT2, (H, H), "outwT2_t")
            outb2_t = wtile(outb2, (128, H), "outb2_t")
            ln1g_t = wtile(ln1g, (128, H), "ln1g_t")
            ln1b_t = wtile(ln1b, (128, H), "ln1b_t")
            ff1wT_t = wtile(ff1wT, (H, H), "ff1wT_t")
            ff1b_t = wtile(ff1b, (128, H), "ff1b_t")
            ff2wT_t = wtile(ff2wT, (H, H), "ff2wT_t")
            ff2b_t = wtile(ff2b, (128, H), "ff2b_t")
            ln2g_t = wtile(ln2g, (128, H), "ln2g_t")
            ln2b_t = wtile(ln2b, (128, H), "ln2b_t")

            ag3_in = dp.tile([GB, H, NROI], FP, name="ag3_in")
            ag3_out = dp.tile([B, H, NROI], FP, name="ag3_out",
                              addr_space="Shared")
            SB2 = NROI - 128  # 20
            CH = ((0, 128), (128, SB2))  # (offset, size) token chunks

            def transpose_pair(dst, srcA, srcB, nm):
                """dst (128, 148) <- [srcA (128,128)]^T cols 0:128,
                [srcB (SB2,128)]^T cols 128:148."""
                tpA = psp.tile([128, 128], FP, name=f"tpA_{nm}", tag="tr", space="PSUM")
                nc.tensor.transpose(tpA[:], srcA, ident[:])
                nc.vector.tensor_copy(dst[:, :128], tpA[:])
                tpB = psp.tile([128, SB2], FP, name=f"tpB_{nm}", tag="tr", space="PSUM")
                nc.tensor.transpose(tpB[:], srcB, ident[:SB2, :SB2])
                nc.vector.tensor_copy(dst[:, 128:], tpB[:])

            def layernorm(xc, sz, gt, bt, nm):
                """in-place LN over free dim of xc (sz,H)."""
                s = wp.tile([128, 1], FP, name=f"s_{nm}", bufs=2)
                nc.vector.reduce_sum(s[:sz], xc, axis=mybir.AxisListType.X)
                nmu = wp.tile([128, 1], FP, name=f"nmu_{nm}", bufs=2)
                nc.vector.tensor_scalar_mul(nmu[:sz], s[:sz], -1.0 / H)
                sqs = wp.tile([128, H], FP, name=f"sqs_{nm}", bufs=2)
                sq = wp.tile([128, 1], FP, name=f"sq_{nm}", bufs=2)
                nc.scalar.activation(sqs[:sz], xc,
                                     mybir.ActivationFunctionType.Square,
                                     accum_out=sq[:sz])
                var = wp.tile([128, 1], FP, name=f"var_{nm}", bufs=2)
                nc.vector.tensor_scalar_mul(var[:sz], sq[:sz], 1.0 / H)
                mu2 = wp.tile([128, 1], FP, name=f"mu2_{nm}", bufs=2)
                nc.vector.tensor_tensor(out=mu2[:sz], in0=nmu[:sz], in1=nmu[:sz],
                                        op=mybir.AluOpType.mult)
                nc.vector.tensor_tensor(out=var[:sz], in0=var[:sz], in1=mu2[:sz],
                                        op=mybir.AluOpType.subtract)
                rstd = wp.tile([128, 1], FP, name=f"rstd_{nm}", bufs=2)
                nc.scalar.activation(rstd[:sz], var[:sz],
                                     mybir.ActivationFunctionType.Sqrt,
                                     bias=eps_t[:sz, :1])
                nc.vector.reciprocal(rstd[:sz], rstd[:sz])
                nc.vector.tensor_scalar(out=xc, in0=xc, scalar1=nmu[:sz, :1],
                                        scalar2=rstd[:sz, :1],
                                        op0=mybir.AluOpType.add,
                                        op1=mybir.AluOpType.mult)
                nc.vector.tensor_tensor(out=xc, in0=xc, in1=gt[:sz],
                                        op=mybir.AluOpType.mult)
                nc.vector.tensor_tensor(out=xc, in0=xc, in1=bt[:sz],
                                        op=mybir.AluOpType.add)

            for g in range(GB):
                cmbT = wp.tile([128, NROI], FP, name="cmbT", bufs=2)
                transpose_pair(cmbT, cmb[:, g, 0], cmb[:SB2, g, 1], "cmb")
                qT2 = wp.tile([128, NROI], FP, name="qT2", bufs=2)
                kT2 = wp.tile([128, NROI], FP, name="kT2", bufs=2)
                pq = psp.tile([128, NROI], FP, name="pq2", tag="acc2", space="PSUM")
                nc.tensor.matmul(pq[:], lhsT=WqT2_t[:], rhs=cmbT[:],
                                 start=True, stop=True)
                nc.scalar.activation(qT2[:], pq[:],
                                     mybir.ActivationFunctionType.Identity,
                                     bias=bq2_t[:, :1])
                pk = psp.tile([128, NROI], FP, name="pk2", tag="acc2", space="PSUM")
                nc.tensor.matmul(pk[:], lhsT=WkT2_t[:], rhs=cmbT[:],
                                 start=True, stop=True)
                nc.scalar.activation(kT2[:], pk[:],
                                     mybir.ActivationFunctionType.Identity,
                                     bias=bk2_t[:, :1])
                # v token-major
                vv = wp.tile([128, 2, H], FP, name="vv", bufs=2)
                for ci, (off, sz) in enumerate(CH):
                    pv = psp.tile([128, H], FP, name="pv2", tag="acc2", space="PSUM")
                    nc.tensor.matmul(pv[:sz], lhsT=cmbT[:, off:off + sz],
                                     rhs=WvT2_t[:], start=True, stop=True)
                    nc.vector.tensor_tensor(out=vv[:sz, ci], in0=pv[:sz],
                                            in1=bv2_t[:sz],
                                            op=mybir.AluOpType.add)
                # attention weights per head + attnT + o
                oo = wp.tile([128, 2, H], FP, name="oo", bufs=2)
                for h in range(NHEADS):
                    hs = slice(h * HD, (h + 1) * HD)
                    wA = wp.tile([128, NROI], FP, name="wA", bufs=2)
                    wB = wp.tile([SB2, NROI], FP, name="wB", bufs=2)
                    for (off, sz), wt_ in zip(CH, (wA, wB)):
                        ps = psp.tile([128, NROI], FP, name="ps2", tag="acc2", space="PSUM")
                        nc.tensor.matmul(ps[:sz], lhsT=qT2[hs, off:off + sz],
                                         rhs=kT2[hs, :], start=True, stop=True,
                                         tile_position=(h * HD, 0))
                        rs = wp.tile([128, 1], FP, name="rs2", bufs=2)
                        nc.scalar.activation(wt_[:sz], ps[:sz],
                                             mybir.ActivationFunctionType.Exp,
                                             bias=0.0, scale=ISCALE,
                                             accum_out=rs[:sz])
                        nc.vector.reciprocal(rs[:sz], rs[:sz])
                        nc.vector.tensor_scalar_mul(wt_[:sz], wt_[:sz], rs[:sz, :1])
                        nc.sync.dma_start(o_attnw[g, h, off:off + sz], wt_[:sz])
                    aT0 = wp.tile([128, NROI], FP, name="aT0", bufs=2)
                    transpose_pair(aT0, wA[:, :128], wB[:, :128], "a0")
                    aT1 = wp.tile([SB2, NROI], FP, name="aT1", bufs=2)
                    tpC = psp.tile([SB2, 128], FP, name="tpC", tag="tr", space="PSUM")
                    nc.tensor.matmul(tpC[:], lhsT=wA[:, 128:], rhs=ident[:],
                                     is_transpose=True, start=True, stop=True)
                    nc.vector.tensor_copy(aT1[:, :128], tpC[:])
                    tpD = psp.tile([SB2, SB2], FP, name="tpD", tag="tr", space="PSUM")
                    nc.tensor.matmul(tpD[:], lhsT=wB[:, 128:],
                                     rhs=ident[:SB2, :SB2],
                                     is_transpose=True, start=True, stop=True)
                    nc.vector.tensor_copy(aT1[:, 128:], tpD[:])
                    for ci, (off, sz) in enumerate(CH):
                        po = psp.tile([128, HD], FP, name="po2", tag="acc2", space="PSUM")
                        nc.tensor.matmul(po[:sz], lhsT=aT0[:, off:off + sz],
                                         rhs=vv[:, 0, hs], start=True, stop=False)
                        nc.tensor.matmul(po[:sz], lhsT=aT1[:, off:off + sz],
                                         rhs=vv[:SB2, 1, hs], start=False,
                                         stop=True)
                        nc.vector.tensor_copy(oo[:sz, ci, hs], po[:sz])
                # out proj + residual + LN1
                ooT = wp.tile([128, NROI], FP, name="ooT", bufs=2)
                transpose_pair(ooT, oo[:, 0], oo[:SB2, 1], "oo")
                x1 = wp.tile([128, 2, H], FP, name="x1", bufs=2)
                for ci, (off, sz) in enumerate(CH):
                    pa = psp.tile([128, H], FP, name="pa2", tag="acc2", space="PSUM")
                    nc.tensor.matmul(pa[:sz], lhsT=ooT[:, off:off + sz],
                                     rhs=outwT2_t[:], start=True, stop=True)
                    nc.vector.tensor_tensor(out=x1[:sz, ci], in0=pa[:sz],
                                            in1=outb2_t[:sz],
                                            op=mybir.AluOpType.add)
                    nc.vector.tensor_tensor(out=x1[:sz, ci], in0=x1[:sz, ci],
                                            in1=cmb[:sz, g, ci],
                                            op=mybir.AluOpType.add)
                    layernorm(x1[:sz, ci], sz, ln1g_t, ln1b_t, "ln1")
                # FFN
                x1T = wp.tile([128, NROI], FP, name="x1T", bufs=2)
                transpose_pair(x1T, x1[:, 0], x1[:SB2, 1], "x1")
                f1 = wp.tile([128, 2, H], FP, name="f1", bufs=2)
                for ci, (off, sz) in enumerate(CH):
                    pf = psp.tile([128, H], FP, name="pf2", tag="acc2", space="PSUM")
                    nc.tensor.matmul(pf[:sz], lhsT=x1T[:, off:off + sz],
                                     rhs=ff1wT_t[:], start=True, stop=True)
                    nc.vector.tensor_tensor(out=f1[:sz, ci], in0=pf[:sz],
                                            in1=ff1b_t[:sz],
                                            op=mybir.AluOpType.add)
                    nc.scalar.activation(f1[:sz, ci], f1[:sz, ci],
                                         mybir.ActivationFunctionType.Relu)
                f1T = wp.tile([128, NROI], FP, name="f1T", bufs=2)
                transpose_pair(f1T, f1[:, 0], f1[:SB2, 1], "f1")
                x3 = wp.tile([128, 2, H], FP, name="x3", bufs=2)
                for ci, (off, sz) in enumerate(CH):
                    pf2 = psp.tile([128, H], FP, name="pf22", tag="acc2", space="PSUM")
                    nc.tensor.matmul(pf2[:sz], lhsT=f1T[:, off:off + sz],
                                     rhs=ff2wT_t[:], start=True, stop=True)
                    nc.vector.tensor_tensor(out=x3[:sz, ci], in0=pf2[:sz],
                                            in1=ff2b_t[:sz],
                                            op=mybir.AluOpType.add)
                    nc.vector.tensor_tensor(out=x3[:sz, ci], in0=x3[:sz, ci],
                                            in1=x1[:sz, ci],
                                            op=mybir.AluOpType.add)
                    layernorm(x3[:sz, ci], sz, ln2g_t, ln2b_t, "ln2")
                    nc.sync.dma_start(o_tout[g, off:off + sz], x3[:sz, ci])
                toutT = wp.tile([128, NROI], FP, name="toutT", bufs=2)
                transpose_pair(toutT, x3[:, 0], x3[:SB2, 1], "to")
                nc.sync.dma_start(ag3_in[g], toutT[:])

            nc.gpsimd.collective_compute(
                "AllGather", mybir.AluOpType.bypass, replica_groups=RG,
                ins=[ag3_in.opt()], outs=[ag3_out.opt()])

            # ---------------- stage 7: MLP + AllReduce
            rhs_all = pp.tile([128, B, NROI], FP, name="rhs_all")
            nc.sync.dma_start(rhs_all[:], ag3_out.rearrange("g f r -> f g r"))
            msc_t = wtile(mlp_scale, (128, 1), "msc_t")
            mbi_t = wtile(mlp_bias, (128, 1), "mbi_t")
            w2sT_t = wtile(w2sT, (128, OUT_D), "w2sT_t")
            phc = psp.tile([128, B], FP, name="phc", tag="acc", space="PSUM")
            NCHUNK = 8
            for j in range(NROI // NCHUNK + 1):
                r0 = j * NCHUNK
                nch = min(NCHUNK, NROI - r0)
                if nch <= 0:
                    break
                w1c = wp.tile([128, NCHUNK * 128], FP, name="w1c")
                nc.sync.dma_start(
                    w1c[:, :nch * 128].rearrange("p (c m) -> p c m", c=nch),
                    w1sT[r0 * 128:(r0 + nch) * 128].rearrange(
                        "(c p) m -> p c m", p=128))
                for cth in range(nch):
                    r = r0 + cth
                    nc.tensor.matmul(
                        phc[:], lhsT=w1c[:, cth * 128:(cth + 1) * 128],
                        rhs=rhs_all[:, :, r], start=(r == 0), stop=(r == NROI - 1))
            zt = wp.tile([128, B], FP, name="zt", bufs=1)
            nc.scalar.activation(zt[:], phc[:],
                                 mybir.ActivationFunctionType.Identity,
                                 bias=mbi_t[:, :1], scale=msc_t[:, :1])
            hc_p = wp.tile([128, B], FP, name="hc_p", bufs=1)
            nc.vector.tensor_scalar_max(hc_p[:], zt[:], 0.0)
            hc_n = wp.tile([128, B], FP, name="hc_n", bufs=1)
            nc.vector.tensor_scalar(out=hc_n[:], in0=zt[:], scalar1=0.0,
                                    scalar2=0.01, op0=mybir.AluOpType.min,
                                    op1=mybir.AluOpType.mult)
            nc.vector.tensor_tensor(out=hc_p[:], in0=hc_p[:], in1=hc_n[:],
                                    op=mybir.AluOpType.add)
            plog = psp.tile([B, OUT_D], FP, name="plog", tag="acc2", space="PSUM")
            nc.tensor.matmul(plog[:], lhsT=hc_p[:], rhs=w2sT_t[:],
                             start=True, stop=True)
            log_sb = wp.tile([B, OUT_D], FP, name="log_sb", bufs=1)
            nc.vector.tensor_copy(log_sb[:], plog[:])
            ar_in = dp.tile([B, OUT_D], FP, name="ar_in")
            ar_out = dp.tile([B, OUT_D], FP, name="ar_out", addr_space="Shared")
            nc.sync.dma_start(ar_in[:], log_sb[:])
            nc.gpsimd.collective_compute(
                "AllReduce", mybir.AluOpType.add, replica_groups=RG,
                ins=[ar_in.opt()], outs=[ar_out.opt()])
            logf = wp.tile([B, OUT_D], FP, name="logf", bufs=1)
            nc.sync.dma_start(logf[:], ar_out[:])
            b2o_t = wtile(b2_out, (B, OUT_D), "b2o_t")
            nc.vector.tensor_tensor(out=logf[:], in0=logf[:], in1=b2o_t[:],
                                    op=mybir.AluOpType.add)
            nc.sync.dma_start(o_logits[:], logf[:])

    nc.compile()
    return nc


# ---------------------------------------------------------------- entry point

def kernel(_trace=False, **inputs):
    in_maps, cfg = _prep(inputs)
    if cfg not in _CACHE:
        _CACHE[cfg] = _build(cfg)
    nc = _CACHE[cfg]
    res = bass_utils.run_bass_kernel_spmd(
        nc, in_maps, core_ids=list(range(NCORE)), trace=_trace)
    kernel.last_results = res
    r = res.results
    cat = lambda k: np.concatenate([r[c][k] for c in range(NCORE)], axis=0)
    out = r[0]["o_logits"]
    embedding = cat("o_emb")
    embedding_roi = cat("o_embroi")
    combined = cat("o_comb")
    transformer_out = cat("o_tout")
    attn_w = cat("o_attnw")
    embedding_3h = cat("o_emb3h")
    w3h = cat("o_w3h")
    return (out, embedding, embedding_roi, combined, transformer_out,
            attn_w, embedding_3h, w3h)


# revision 14
# speedup vs baseline: 1.5377x; 1.5377x over previous
"""Trainium2 Bass kernel for nn_ModelGCNAttn3h (GCN + attention + ROI pooling).

Self-contained: hardcodes shapes/sharding. kernel(**inputs) takes full inputs,
shards across 8 NeuronCores (2 graphs per core), runs one SPMD NEFF, gathers.

Sharding: data-parallel by graph (2 per core). GCN edges sharded by dst node;
layer-1 source features are host-pre-gathered (edge-major layout), layer-2
gathers from an all-gathered bf16 table via indirect DMA. Segment reductions
are one-hot matmuls on the TensorEngine (bf16, fp32 accumulate).
"""
import math

import ml_dtypes
import numpy as np

import concourse.bacc as bacc
import concourse.bass as bass
import concourse.mybir as mybir
import concourse.tile as tile
from concourse import bass_utils
from concourse.masks import make_identity

FP = mybir.dt.float32
BF = mybir.dt.bfloat16
I32 = mybir.dt.int32
NPBF = ml_dtypes.bfloat16

B = 16
NPER = 1024
N = B * NPER
NROI = 148
N2 = B * NROI
DIN = 64
H = 128
NHEADS = 4
HD = 32
OUT_D = 2
EPS = 1e-5
NCORE = 8
GB = B // NCORE            # graphs per core = 2
NLV = GB * NPER            # 2048 vertex nodes per core
NLR = GB * NROI            # 296 roi nodes per core
NWV = NLV // 128           # 16 vertex windows
NWR = math.ceil(NLR / 128)  # 3 roi windows
G = 4                      # 128-edge blocks fused per DVE op
BLK = 128
ISCALE = 1.0 / math.sqrt(HD)
SB2 = NROI - 128           # 20

_CACHE = {}


# ---------------------------------------------------------------- host prep

def _shard_edges(src, dst, w, x_nodes, n_nodes, npc, nwin, srow_of):
    """Sort edges (incl self loops) by (core, dst window); pad per-window block
    counts to a global (multiple-of-G) max. Returns per-core arrays:
      e_idx (NSB,128,G) i32  gather row (global src id via srow_of)
      e_w   (NSB,128,G) f32  raw edge weight
      e_ld  (NSB,128,G) bf16 local dst within window (pad -1)
      x_g   (NBLK,128,DIN) bf16  pre-gathered source features (layer 1)
      edegw (NBLK,128,dcap) f32  in-edge weights of each edge's SOURCE node
      degw  (nwin*128, dcap) f32 per-core dst-side deg table (pads deg=1)
    """
    src = np.concatenate([src.astype(np.int64), np.arange(n_nodes, dtype=np.int64)])
    dst = np.concatenate([dst.astype(np.int64), np.arange(n_nodes, dtype=np.int64)])
    w = np.concatenate([np.asarray(w, np.float32), np.ones(n_nodes, np.float32)])
    core = dst // npc
    dloc = dst - core * npc
    win = dloc // BLK
    ldst = (dloc % BLK).astype(np.float32)
    key = core * nwin + win
    cnt = np.bincount(key, minlength=NCORE * nwin).reshape(NCORE, nwin)
    nbw = (np.maximum(np.ceil(cnt.max(0) / (BLK * G)), 1) * G).astype(int)
    nblk = int(nbw.sum())
    srow = srow_of(src)

    # global node-major in-edge weight table (for both dst-deg and src-deg)
    dcap = int(np.bincount(dst, minlength=n_nodes).max())
    dcap = ((dcap + 3) // 4) * 4
    node_degw = np.zeros((n_nodes + 1, dcap), np.float32)  # row n_nodes: pad
    node_degw[n_nodes, 0] = 1.0
    ordd = np.argsort(dst, kind="stable")
    dcnt = np.bincount(dst, minlength=n_nodes)
    dstart = np.zeros(n_nodes + 1, np.int64)
    np.cumsum(dcnt, out=dstart[1:])
    pos = np.arange(len(dst)) - dstart[dst[ordd]]
    node_degw[dst[ordd], pos] = w[ordd]

    order = np.argsort(key, kind="stable")
    starts = np.zeros(NCORE * nwin + 1, np.int64)
    np.cumsum(cnt.reshape(-1), out=starts[1:])
    woff = np.zeros(nwin + 1, np.int64)
    np.cumsum(nbw * BLK, out=woff[1:])

    e_idx = np.zeros((NCORE, nblk * BLK), np.int32)
    e_w = np.zeros((NCORE, nblk * BLK), np.float32)
    e_ld = np.full((NCORE, nblk * BLK), -1.0, np.float32)
    e_src = np.full((NCORE, nblk * BLK), n_nodes, np.int64)  # pad -> deg-1 row
    for c in range(NCORE):
        for wi in range(nwin):
            s, e = starts[c * nwin + wi], starts[c * nwin + wi + 1]
            sel = order[s:e]
            o = woff[wi]
            e_idx[c, o:o + len(sel)] = srow[sel]
            e_w[c, o:o + len(sel)] = w[sel]
            e_ld[c, o:o + len(sel)] = ldst[sel]
            e_src[c, o:o + len(sel)] = src[sel]

    def to_sb(a):  # (nblk*128,) -> (NSB, 128, G)
        return np.ascontiguousarray(
            a.reshape(nblk // G, G, BLK).transpose(0, 2, 1))

    x_pad = np.vstack([np.asarray(x_nodes, np.float32),
                       np.zeros((1, x_nodes.shape[1]), np.float32)])
    out = dict(
        e_idx=np.stack([to_sb(e_idx[c]) for c in range(NCORE)]),
        e_w=np.stack([to_sb(e_w[c]) for c in range(NCORE)]),
        e_ld=np.stack([to_sb(e_ld[c]).astype(NPBF) for c in range(NCORE)]),
        x_g=np.stack([
            x_pad[np.minimum(e_src[c], n_nodes)].reshape(nblk, BLK, -1)
            .astype(NPBF) for c in range(NCORE)]),
        edegw=np.stack([
            node_degw[e_src[c]].reshape(nblk, BLK, dcap) for c in range(NCORE)]),
        degw=np.stack([
            np.vstack([node_degw[c * npc:(c + 1) * npc],
                       np.broadcast_to(node_degw[n_nodes], (nwin * BLK - npc, dcap))])
            for c in range(NCORE)]),
        nbw=[int(v) for v in nbw], dcap=dcap,
    )
    return out


def _prep(inputs):
    x = np.asarray(inputs["x"], np.float32)
    node_roi = np.asarray(inputs["node_roi"], np.int64)
    batch = np.asarray(inputs["batch"], np.int64)
    ei = np.asarray(inputs["edge_index"], np.int64)
    ew = np.asarray(inputs["edge_weight"], np.float32)
    x2 = np.asarray(inputs["x2"], np.float32)
    roi2 = np.asarray(inputs["roi_label2"], np.int64)
    batch2 = np.asarray(inputs["batch2"], np.int64)
    ei2 = np.asarray(inputs["edge_index2"], np.int64)
    ew2 = np.asarray(inputs["edge_weight2"], np.float32)
    assert np.array_equal(batch, np.repeat(np.arange(B), NPER)), "non-uniform batch"
    assert np.array_equal(batch2, np.repeat(np.arange(B), NROI)), "non-uniform batch2"
    assert np.array_equal(roi2, np.tile(np.arange(NROI), B)), "roi_label2 not arange"

    ev = _shard_edges(ei[0], ei[1], ew, x, N, NLV, NWV, lambda s: s.astype(np.int32))
    er = _shard_edges(ei2[0], ei2[1], ew2, x2, N2, NLR, NWR,
                      lambda s: s.astype(np.int32))

    pool_roi = node_roi.reshape(NCORE, NWV, BLK).astype(NPBF)
    pool_cnt = np.zeros((NCORE, GB, NROI), np.float32)
    for g in range(B):
        pool_cnt[g // GB, g % GB] = np.bincount(
            node_roi[g * NPER:(g + 1) * NPER], minlength=NROI)

    p3 = {k: np.asarray(v, np.float32) for k, v in inputs["params_mha3h"].items()}
    p2 = {k: np.asarray(v, np.float32) for k, v in inputs["params_attn_sum"].items()}
    pm = {k: np.asarray(v, np.float32) for k, v in inputs["params_mlp"].items()}
    gv = [(np.asarray(W, np.float32), np.asarray(b, np.float32))
          for W, b in inputs["params_gcn"]]
    gr = [(np.asarray(W, np.float32), np.asarray(b, np.float32))
          for W, b in inputs["params_gcn_roi"]]

    rep = lambda v: np.ascontiguousarray(np.tile(v.reshape(1, -1), (128, 1)))
    col = lambda v: np.ascontiguousarray(v.reshape(-1, 1))
    bft = lambda a: np.ascontiguousarray(a.T).astype(NPBF)
    shared = {
        "W1_v": gv[0][0], "b1_v_rep": rep(gv[0][1]),
        "W2_v": gv[1][0], "b2_v_rep": rep(gv[1][1]),
        "W1_r": gr[0][0], "b1_r_rep": rep(gr[0][1]),
        "W2_r": gr[1][0], "b2_r_rep": rep(gr[1][1]),
        "Wq3T": bft(p3["in_w"][:H]), "Wk3T": bft(p3["in_w"][H:2 * H]),
        "bq3": col(p3["in_b"][:H]), "bk3": col(p3["in_b"][H:2 * H]),
        "WqT2": bft(p2["in_w"][:H]), "WkT2": bft(p2["in_w"][H:2 * H]),
        "WvT2": bft(p2["in_w"][2 * H:]),
        "bq2": col(p2["in_b"][:H]), "bk2": col(p2["in_b"][H:2 * H]),
        "bv2_rep": rep(p2["in_b"][2 * H:]),
        "outwT2": bft(p2["out_w"]), "outb2_rep": rep(p2["out_b"]),
        "ln1g_rep": rep(p2["ln1_g"]), "ln1b_rep": rep(p2["ln1_b"]),
        "ff1wT": bft(p2["ff1_w"]), "ff1b_rep": rep(p2["ff1_b"]),
        "ff2wT": bft(p2["ff2_w"]), "ff2b_rep": rep(p2["ff2_b"]),
        "ln2g_rep": rep(p2["ln2_g"]), "ln2b_rep": rep(p2["ln2_b"]),
        "b2_out_rep": np.ascontiguousarray(np.tile(pm["b2"].reshape(1, -1), (B, 1))),
    }

    NSL = 1000 // NCORE
    bn_scale = pm["bn_g"] / math.sqrt(1.0 + EPS)
    bn_bias = pm["b1"] * bn_scale + pm["bn_b"]
    in_maps = []
    for c in range(NCORE):
        w1s = pm["w1"][c * NSL:(c + 1) * NSL]
        w1sT = np.zeros((NROI * H, 128), NPBF)
        w1sT[:, :NSL] = w1s.T.astype(NPBF)
        w2sT = np.zeros((128, OUT_D), np.float32)
        w2sT[:NSL] = pm["w2"][:, c * NSL:(c + 1) * NSL].T
        msc = np.zeros((128, 1), np.float32)
        msc[:NSL, 0] = bn_scale[c * NSL:(c + 1) * NSL]
        mbi = np.zeros((128, 1), np.float32)
        mbi[:NSL, 0] = bn_bias[c * NSL:(c + 1) * NSL]
        m = {
            "degw_v": ev["degw"][c], "degw_r": er["degw"][c],
            "e_idx_v": ev["e_idx"][c], "e_w_v": ev["e_w"][c],
            "e_ldst_v": ev["e_ld"][c], "xg_v": ev["x_g"][c],
            "edegw_v": ev["edegw"][c],
            "e_idx_r": er["e_idx"][c], "e_w_r": er["e_w"][c],
            "e_ldst_r": er["e_ld"][c], "xg_r": er["x_g"][c],
            "edegw_r": er["edegw"][c],
            "pool_roi": pool_roi[c], "pool_cnt": pool_cnt[c],
            "w1sT": w1sT, "w2sT": w2sT, "mlp_scale": msc, "mlp_bias": mbi,
        }
        m.update(shared)
        in_maps.append(m)

    cfg = (tuple(ev["nbw"]), tuple(er["nbw"]), ev["dcap"], er["dcap"])
    return in_maps, cfg


# ---------------------------------------------------------------- device build

def _build(cfg):
    nbw_v, nbw_r, dcap_v, dcap_r = cfg
    nblk_v, nblk_r = sum(nbw_v), sum(nbw_r)
    nsb_v, nsb_r = nblk_v // G, nblk_r // G

    nc = bacc.Bacc("TRN2", target_bir_lowering=False, debug=False,
                   num_devices=NCORE)

    def di(name, shape, dt=FP):
        return nc.dram_tensor(name, list(shape), dt, kind="ExternalInput").ap()

    def do(name, shape, dt=FP):
        return nc.dram_tensor(name, list(shape), dt, kind="ExternalOutput").ap()

    degw_v = di("degw_v", (NWV * BLK, dcap_v))
    degw_r = di("degw_r", (NWR * BLK, dcap_r))
    e_idx_v = di("e_idx_v", (nsb_v, BLK, G), I32)
    e_w_v = di("e_w_v", (nsb_v, BLK, G))
    e_ld_v = di("e_ldst_v", (nsb_v, BLK, G), BF)
    xg_v_d = di("xg_v", (nblk_v, BLK, DIN), BF)
    edegw_v = di("edegw_v", (nblk_v, BLK, dcap_v))
    e_idx_r = di("e_idx_r", (nsb_r, BLK, G), I32)
    e_w_r = di("e_w_r", (nsb_r, BLK, G))
    e_ld_r = di("e_ldst_r", (nsb_r, BLK, G), BF)
    xg_r_d = di("xg_r", (nblk_r, BLK, DIN), BF)
    edegw_r = di("edegw_r", (nblk_r, BLK, dcap_r))
    pool_roi = di("pool_roi", (NWV, BLK), BF)
    pool_cnt = di("pool_cnt", (GB, NROI))
    w1sT = di("w1sT", (NROI * H, 128), BF)
    w2sT = di("w2sT", (128, OUT_D))
    mlp_scale = di("mlp_scale", (128, 1))
    mlp_bias = di("mlp_bias", (128, 1))

    W1_v = di("W1_v", (DIN, H)); b1_v = di("b1_v_rep", (128, H))
    W2_v = di("W2_v", (H, H)); b2_v = di("b2_v_rep", (128, H))
    W1_r = di("W1_r", (DIN, H)); b1_r = di("b1_r_rep", (128, H))
    W2_r = di("W2_r", (H, H)); b2_r = di("b2_r_rep", (128, H))
    Wq3T = di("Wq3T", (H, H), BF); Wk3T = di("Wk3T", (H, H), BF)
    bq3 = di("bq3", (H, 1)); bk3 = di("bk3", (H, 1))
    WqT2 = di("WqT2", (H, H), BF); WkT2 = di("WkT2", (H, H), BF)
    WvT2 = di("WvT2", (H, H), BF)
    bq2 = di("bq2", (H, 1)); bk2 = di("bk2", (H, 1)); bv2 = di("bv2_rep", (128, H))
    outwT2 = di("outwT2", (H, H), BF); outb2 = di("outb2_rep", (128, H))
    ln1g = di("ln1g_rep", (128, H)); ln1b = di("ln1b_rep", (128, H))
    ff1wT = di("ff1wT", (H, H), BF); ff1b = di("ff1b_rep", (128, H))
    ff2wT = di("ff2wT", (H, H), BF); ff2b = di("ff2b_rep", (128, H))
    ln2g = di("ln2g_rep", (128, H)); ln2b = di("ln2b_rep", (128, H))
    b2_out = di("b2_out_rep", (B, OUT_D))

    o_logits = do("o_logits", (B, OUT_D))
    o_emb = do("o_emb", (GB, NROI, H))
    o_embroi = do("o_embroi", (GB, NROI, H))
    o_comb = do("o_comb", (GB, NROI, H))
    o_tout = do("o_tout", (GB, NROI, H))
    o_attnw = do("o_attnw", (GB, NHEADS, NROI, NROI))
    o_emb3h = do("o_emb3h", (NLV, H))
    o_w3h = do("o_w3h", (GB, NHEADS, NPER, NPER))

    RG = [list(range(NCORE))]

    with tile.TileContext(nc) as tc:
        with (
            tc.tile_pool(name="persist", bufs=1) as pp,
            tc.tile_pool(name="work", bufs=3) as wp,
            tc.tile_pool(name="psum", bufs=2, space="PSUM") as psp,
            tc.tile_pool(name="dram", bufs=1, space="DRAM") as dp,
        ):
            ident = pp.tile([128, 128], FP, name="ident")
            make_identity(nc, ident[:])
            iota128 = pp.tile([128, G * 128], BF, name="iota128")
            iota128_i = pp.tile([128, G * 128], I32, name="iota128_i")
            nc.gpsimd.iota(iota128_i[:].rearrange("p (g w) -> p g w", g=G),
                           pattern=[[0, G], [1, 128]], base=0, channel_multiplier=0)
            nc.vector.tensor_copy(iota128[:], iota128_i[:])
            iota148 = pp.tile([128, NROI], BF, name="iota148")
            iota148_i = pp.tile([128, NROI], I32, name="iota148_i")
            nc.gpsimd.iota(iota148_i[:], pattern=[[1, NROI]], base=0,
                           channel_multiplier=0)
            nc.vector.tensor_copy(iota148[:], iota148_i[:])
            eps_t = pp.tile([128, 1], FP, name="eps_t")
            nc.gpsimd.memset(eps_t[:], EPS)

            # ---------------- deg / dinv (dst side) and per-edge src dinv
            def deg_stage(degw_d, nwin, dcap, nm):
                dw, dw_free = tc.tile([128, nwin, dcap], FP, name=f"dw_{nm}")
                nc.sync.dma_start(
                    dw[:], degw_d.rearrange("(w p) c -> p w c", p=128))
                deg = pp.tile([128, nwin], FP, name=f"deg_{nm}")
                nc.vector.reduce_sum(deg[:].unsqueeze(-1), dw[:],
                                     axis=mybir.AxisListType.X)
                nc.scalar.sqrt(deg[:], deg[:])
                dinv = pp.tile([128, nwin], FP, name=f"dinv_{nm}")
                nc.vector.reciprocal(dinv[:], deg[:])
                dw_free()
                return dinv

            dinv_v = deg_stage(degw_v, NWV, dcap_v, "v")
            dinv_r = deg_stage(degw_r, NWR, dcap_r, "r")

            def edge_scale(edegw_d, e_w_d, nblk, dcap, nm):
                """ewt = raw w_e; wc = w_e / sqrt(deg[src_e]). Both (128,nsb,G)."""
                nsb = nblk // G
                ewt = pp.tile([128, nsb, G], FP, name=f"ewt_{nm}")
                nc.sync.dma_start(ewt[:], e_w_d.rearrange("s p g -> p s g"))
                wc = pp.tile([128, nsb, G], FP, name=f"wc_{nm}")
                CH = 32  # blocks per chunk
                with tc.tile_pool(name=f"edsc_{nm}", bufs=2) as lp:
                    for j in range(0, nblk, CH):
                        n = min(CH, nblk - j)
                        ed = lp.tile([128, CH, dcap], FP, name=f"ed_{nm}")
                        nc.sync.dma_start(
                            ed[:, :n], edegw_d[j:j + n].rearrange("b p c -> p b c"))
                        es = lp.tile([128, CH], FP, name=f"es_{nm}")
                        nc.vector.reduce_sum(es[:, :n].unsqueeze(-1), ed[:, :n],
                                             axis=mybir.AxisListType.X)
                        nc.scalar.sqrt(es[:, :n], es[:, :n])
                        er_ = lp.tile([128, CH], FP, name=f"er_{nm}")
                        nc.vector.reciprocal(er_[:, :n], es[:, :n])
                        nc.vector.tensor_tensor(
                            out=wc[:].rearrange("p s g -> p (s g)")[:, j:j + n],
                            in0=ewt[:].rearrange("p s g -> p (s g)")[:, j:j + n],
                            in1=er_[:, :n], op=mybir.AluOpType.mult)
                return ewt, wc

            ewt_v, wc_v = edge_scale(edegw_v, e_w_v, nblk_v, dcap_v, "v")
            ewt_r, wc_r = edge_scale(edegw_r, e_w_r, nblk_r, dcap_r, "r")

            # edge metadata + layer-1 pre-gathered features to SBUF
            def load_edges(e_idx_d, e_ld_d, xg_d, nsb, nblk, nm):
                it = pp.tile([128, nsb, G], I32, name=f"eidx_{nm}")
                lt = pp.tile([128, nsb, G], BF, name=f"eld_{nm}")
                nc.sync.dma_start(it[:], e_idx_d.rearrange("s p g -> p s g"))
                nc.sync.dma_start(lt[:], e_ld_d.rearrange("s p g -> p s g"))
                xg = pp.tile([128, nblk, DIN], BF, name=f"xg_{nm}")
                CH = 64
                for j in range(0, nblk, CH):
                    n = min(CH, nblk - j)
                    nc.sync.dma_start(
                        xg[:, j:j + n], xg_d[j:j + n].rearrange("b p d -> p b d"))
                return it, lt, xg

            ev_t = load_edges(e_idx_v, e_ld_v, xg_v_d, nsb_v, nblk_v, "v")
            er_t = load_edges(e_idx_r, e_ld_r, xg_r_d, nsb_r, nblk_r, "r")

            def wtile(d, shape, nm, dt=FP):
                t = pp.tile(list(shape), dt, name=nm)
                nc.sync.dma_start(t[:], d[:])
                return t

            W1v_t = wtile(W1_v, (DIN, H), "W1v_t")
            b1v_t = wtile(b1_v, (128, H), "b1v_t")
            W2v_t = wtile(W2_v, (H, H), "W2v_t")
            b2v_t = wtile(b2_v, (128, H), "b2v_t")
            W1r_t = wtile(W1_r, (DIN, H), "W1r_t")
            b1r_t = wtile(b1_r, (128, H), "b1r_t")
            W2r_t = wtile(W2_r, (H, H), "W2r_t")
            b2r_t = wtile(b2_r, (128, H), "b2r_t")

            # ---------------- GCN sweep helper
            def onehot_for(oh, lt, sb):
                nc.vector.tensor_tensor(
                    out=oh[:].rearrange("p (g w) -> p g w", g=G),
                    in0=lt[:, sb].unsqueeze(-1).broadcast_to([128, G, 128]),
                    in1=iota128[:].rearrange("p (g w) -> p g w", g=G),
                    op=mybir.AluOpType.is_equal)

            def gcn_finalize(nm, wi, agg, din, dinv, Wt, brt, out_cb):
                aggs = wp.tile([128, din], FP, name=f"aggs_{nm}")
                nc.scalar.activation(aggs[:], agg[:],
                                     mybir.ActivationFunctionType.Copy,
                                     bias=0.0, scale=dinv[:, wi:wi + 1])
                tp = psp.tile([din, 128], FP, name=f"tp_{nm}", tag="tr",
                              space="PSUM")
                nc.tensor.transpose(tp[:], aggs[:], ident[:])
                aggT = wp.tile([din, 128], FP, name=f"aggT_{nm}")
                nc.vector.tensor_copy(aggT[:], tp[:])
                hp = psp.tile([128, H], FP, name=f"hp_{nm}", tag="acc2",
                              space="PSUM")
                nc.tensor.matmul(hp[:], lhsT=aggT[:], rhs=Wt[:],
                                 start=True, stop=True)
                pre = wp.tile([128, H], FP, name=f"pre_{nm}")
                nc.vector.tensor_tensor(out=pre[:], in0=hp[:], in1=brt[:],
                                        op=mybir.AluOpType.add)
                out_cb(wi, pre)

            def gcn_l1(nm, edges, wc, nbw, dinv, Wt, brt, out_cb):
                it, lt, xg = edges
                sb0 = 0
                for wi, nb in enumerate(nbw):
                    agg = psp.tile([128, DIN], FP, name=f"agg1_{nm}", tag="acc",
                                   space="PSUM")
                    for k in range(nb // G):
                        sb = sb0 + k
                        ft = wp.tile([128, G * DIN], BF, name=f"ft1_{nm}")
                        nc.vector.tensor_tensor(
                            out=ft[:].rearrange("p (g d) -> p g d", g=G),
                            in0=xg[:, sb * G:(sb + 1) * G],
                            in1=wc[:, sb].unsqueeze(-1).broadcast_to(
                                [128, G, DIN]),
                            op=mybir.AluOpType.mult)
                        oh = wp.tile([128, G * 128], BF, name=f"oh1_{nm}")
                        onehot_for(oh, lt, sb)
                        for g in range(G):
                            nc.tensor.matmul(
                                agg[:], lhsT=oh[:, g * 128:(g + 1) * 128],
                                rhs=ft[:, g * DIN:(g + 1) * DIN],
                                start=(k == 0 and g == 0),
                                stop=(k == nb // G - 1 and g == G - 1))
                    sb0 += nb // G
                    gcn_finalize(nm, wi, agg, DIN, dinv, Wt, brt, out_cb)

            def gcn_l2(nm, edges, e_w_wc, table, nbw, dinv, Wt, brt, out_cb):
                it, lt, xg = edges
                sb0 = 0
                for wi, nb in enumerate(nbw):
                    agg = psp.tile([128, H], FP, name=f"agg2_{nm}", tag="acc",
                                   space="PSUM")
                    for k in range(nb // G):
                        sb = sb0 + k
                        gt = wp.tile([128, G * H], BF, name=f"gt2_{nm}")
                        for g in range(G):
                            nc.gpsimd.indirect_dma_start(
                                out=gt[:, g * H:(g + 1) * H],
                                out_offset=None, in_=table[:],
                                in_offset=bass.IndirectOffsetOnAxis(
                                    ap=it[:, sb, g:g + 1], axis=0))
                        nc.vector.tensor_tensor(
                            out=gt[:].rearrange("p (g d) -> p g d", g=G),
                            in0=gt[:].rearrange("p (g d) -> p g d", g=G),
                            in1=e_w_wc[:, sb].unsqueeze(-1).broadcast_to(
                                [128, G, H]),
                            op=mybir.AluOpType.mult)
                        oh = wp.tile([128, G * 128], BF, name=f"oh2_{nm}")
                        onehot_for(oh, lt, sb)
                        for g in range(G):
                            nc.tensor.matmul(
                                agg[:], lhsT=oh[:, g * 128:(g + 1) * 128],
                                rhs=gt[:, g * H:(g + 1) * H],
                                start=(k == 0 and g == 0),
                                stop=(k == nb // G - 1 and g == G - 1))
                    sb0 += nb // G
                    gcn_finalize(nm, wi, agg, H, dinv, Wt, brt, out_cb)

            # ---------------- layer 1 -> bf16 h1p tables (AllGather)
            ag2v_in = dp.tile([NLV, H], BF, name="ag2v_in")
            vtab = dp.tile([N, H], BF, name="vtab", addr_space="Shared")
            ag2r_in = dp.tile([NLR, H], BF, name="ag2r_in")
            rtab = dp.tile([N2, H], BF, name="rtab", addr_space="Shared")

            def l1v_cb(wi, pre):
                h1p = wp.tile([128, H], BF, name="h1p_v")
                nc.scalar.activation(h1p[:], pre[:],
                                     mybir.ActivationFunctionType.Relu,
                                     bias=0.0, scale=dinv_v[:, wi:wi + 1])
                nc.sync.dma_start(ag2v_in[wi * BLK:(wi + 1) * BLK], h1p[:])

            gcn_l1("v1", ev_t, wc_v, nbw_v, dinv_v, W1v_t, b1v_t, l1v_cb)
            nc.gpsimd.collective_compute(
                "AllGather", mybir.AluOpType.bypass, replica_groups=RG,
                ins=[ag2v_in.opt()], outs=[vtab.opt()])

            def l1r_cb(wi, pre):
                h1p = wp.tile([128, H], BF, name="h1p_r")
                nc.scalar.activation(h1p[:], pre[:],
                                     mybir.ActivationFunctionType.Relu,
                                     bias=0.0, scale=dinv_r[:, wi:wi + 1])
                lo = wi * BLK
                sz = min(BLK, NLR - lo)
                nc.sync.dma_start(ag2r_in[lo:lo + sz], h1p[:sz])

            gcn_l1("r1", er_t, wc_r, nbw_r, dinv_r, W1r_t, b1r_t, l1r_cb)
            nc.gpsimd.collective_compute(
                "AllGather", mybir.AluOpType.bypass, replica_groups=RG,
                ins=[ag2r_in.opt()], outs=[rtab.opt()])

            # ---------------- layer 2 -> h2 (vertex, f32) / h2r (roi)
            h2 = pp.tile([128, NWV, H], FP, name="h2")
            h2c = pp.tile([128, NWV, H], BF, name="h2c")

            def l2v_cb(wi, pre):
                nc.scalar.activation(h2[:, wi], pre[:],
                                     mybir.ActivationFunctionType.Relu)
                nc.vector.tensor_copy(h2c[:, wi], h2[:, wi])
                nc.sync.dma_start(o_emb3h[wi * BLK:(wi + 1) * BLK], h2[:, wi])

            gcn_l2("v2", ev_t, ewt_v, vtab, nbw_v, dinv_v, W2v_t, b2v_t, l2v_cb)

            embroi_dram = dp.tile([NLR, H], FP, name="embroi_dram")

            def l2r_cb(wi, pre):
                h2r = wp.tile([128, H], FP, name="h2r")
                nc.scalar.activation(h2r[:], pre[:],
                                     mybir.ActivationFunctionType.Relu)
                lo = wi * BLK
                sz = min(BLK, NLR - lo)
                nc.sync.dma_start(embroi_dram[lo:lo + sz], h2r[:sz])

            gcn_l2("r2", er_t, ewt_r, rtab, nbw_r, dinv_r, W2r_t, b2r_t, l2r_cb)
            nc.sync.dma_start(o_embroi[:].rearrange("g r h -> (g r) h"),
                              embroi_dram[:])

            # ---------------- ROI-mean pooling (no gather) + combined
            proi_t = pp.tile([128, NWV], BF, name="proi_t")
            nc.sync.dma_start(proi_t[:], pool_roi.rearrange("w p -> p w"))
            cnt_t = pp.tile([128, GB, 2], FP, name="cnt_t")
            nc.gpsimd.memset(cnt_t[:], 1.0)
            for g in range(GB):
                nc.sync.dma_start(cnt_t[:, g, 0].unsqueeze(-1),
                                  pool_cnt[g, :128].unsqueeze(-1))
                nc.sync.dma_start(cnt_t[:SB2, g, 1].unsqueeze(-1),
                                  pool_cnt[g, 128:].unsqueeze(-1))
            nc.vector.tensor_scalar_max(cnt_t[:], cnt_t[:], 1.0)
            rcnt_t = pp.tile([128, GB, 2], FP, name="rcnt_t")
            nc.vector.reciprocal(rcnt_t[:], cnt_t[:])

            cmb = pp.tile([128, GB, 2, H], FP, name="cmb")
            for g in range(GB):
                psA = psp.tile([128, H], FP, name="psA", tag="acc", space="PSUM")
                psB = psp.tile([SB2, H], FP, name="psB", tag="acc2", space="PSUM")
                for wi in range(8):
                    w_ = g * 8 + wi
                    oh = wp.tile([128, NROI], BF, name="oh_p")
                    nc.vector.tensor_tensor(
                        out=oh[:],
                        in0=proi_t[:, w_].unsqueeze(-1).broadcast_to(
                            [128, NROI]),
                        in1=iota148[:], op=mybir.AluOpType.is_equal)
                    nc.tensor.matmul(psA[:], lhsT=oh[:, :128], rhs=h2c[:, w_],
                                     start=(wi == 0), stop=(wi == 7),
                                     skip_group_check=True)
                    nc.tensor.matmul(psB[:], lhsT=oh[:, 128:], rhs=h2c[:, w_],
                                     start=(wi == 0), stop=(wi == 7),
                                     skip_group_check=True)
                embA = wp.tile([128, H], FP, name="embA", bufs=2)
                nc.scalar.activation(embA[:], psA[:],
                                     mybir.ActivationFunctionType.Copy,
                                     bias=0.0, scale=rcnt_t[:, g, 0].unsqueeze(-1))
                embB = wp.tile([SB2, H], FP, name="embB", bufs=2)
                nc.scalar.activation(embB[:], psB[:],
                                     mybir.ActivationFunctionType.Copy,
                                     bias=0.0,
                                     scale=rcnt_t[:SB2, g, 1].unsqueeze(-1))
                nc.sync.dma_start(o_emb[g, :128], embA[:])
                nc.sync.dma_start(o_emb[g, 128:], embB[:])
                roiA = wp.tile([128, H], FP, name="roiA", bufs=2)
                nc.sync.dma_start(roiA[:], embroi_dram[g * NROI:g * NROI + 128])
                roiB = wp.tile([SB2, H], FP, name="roiB", bufs=2)
                nc.sync.dma_start(roiB[:], embroi_dram[g * NROI + 128:(g + 1) * NROI])
                nc.vector.tensor_tensor(out=cmb[:, g, 0], in0=embA[:], in1=roiA[:],
                                        op=mybir.AluOpType.add)
                nc.vector.tensor_tensor(out=cmb[:SB2, g, 1], in0=embB[:],
                                        in1=roiB[:], op=mybir.AluOpType.add)
                nc.sync.dma_start(o_comb[g, :128], cmb[:, g, 0])
                nc.sync.dma_start(o_comb[g, 128:], cmb[:SB2, g, 1])

            # ---------------- 3h attention weights (w3h)
            Wq3_t = wtile(Wq3T, (H, H), "Wq3_t", BF)
            Wk3_t = wtile(Wk3T, (H, H), "Wk3_t", BF)
            bq3_t = wtile(bq3, (H, 1), "bq3_t")
            bk3_t = wtile(bk3, (H, 1), "bk3_t")
            for g in range(GB):
                h2T = wp.tile([128, NPER], BF, name="h2T", bufs=2)
                for wi in range(8):
                    tp = psp.tile([128, 128], FP, name="tp_a3", tag="tr",
                                  space="PSUM")
                    nc.tensor.transpose(tp[:], h2[:, g * 8 + wi], ident[:])
                    nc.vector.tensor_copy(h2T[:, wi * 128:(wi + 1) * 128], tp[:])
                qT = wp.tile([128, NPER], BF, name="qT", bufs=2)
                kT = wp.tile([128, NPER], BF, name="kT", bufs=2)
                for half in range(2):
                    s = slice(half * 512, (half + 1) * 512)
                    pq = psp.tile([128, 512], FP, name="pq", tag="acc2",
                                  space="PSUM")
                    nc.tensor.matmul(pq[:], lhsT=Wq3_t[:], rhs=h2T[:, s],
                                     start=True, stop=True)
                    nc.scalar.activation(qT[:, s], pq[:],
                                         mybir.ActivationFunctionType.Identity,
                                         bias=bq3_t[:, :1])
                    pk = psp.tile([128, 512], FP, name="pk", tag="acc2",
                                  space="PSUM")
                    nc.tensor.matmul(pk[:], lhsT=Wk3_t[:], rhs=h2T[:, s],
                                     start=True, stop=True)
                    nc.scalar.activation(kT[:, s], pk[:],
                                         mybir.ActivationFunctionType.Identity,
                                         bias=bk3_t[:, :1])
                for h in range(NHEADS):
                    hs = slice(h * HD, (h + 1) * HD)
                    for qi in range(NPER // 128):
                        ps = psp.tile([128, NPER], FP, name="ps_sc", tag="acc",
                                      space="PSUM")
                        for half in range(2):
                            nc.tensor.matmul(
                                ps[:, half * 512:(half + 1) * 512],
                                lhsT=qT[hs, qi * 128:(qi + 1) * 128],
                                rhs=kT[hs, half * 512:(half + 1) * 512],
                                start=True, stop=True,
                                tile_position=(h * HD, 0))
                        et = wp.tile([128, NPER], FP, name="et", bufs=2)
                        rs = wp.tile([128, 1], FP, name="rs")
                        nc.scalar.activation(et[:], ps[:],
                                             mybir.ActivationFunctionType.Exp,
                                             bias=0.0, scale=ISCALE,
                                             accum_out=rs[:])
                        nc.vector.reciprocal(rs[:], rs[:])
                        nc.vector.tensor_scalar_mul(et[:], et[:], rs[:, :1])
                        nc.sync.dma_start(
                            o_w3h[g, h, qi * 128:(qi + 1) * 128], et[:])

            # ---------------- summary attention block (attn2)
            WqT2_t = wtile(WqT2, (H, H), "WqT2_t", BF)
            WkT2_t = wtile(WkT2, (H, H), "WkT2_t", BF)
            WvT2_t = wtile(WvT2, (H, H), "WvT2_t", BF)
            bq2_t = wtile(bq2, (H, 1), "bq2_t")
            bk2_t = wtile(bk2, (H, 1), "bk2_t")
            bv2_t = wtile(bv2, (128, H), "bv2_t")
            outwT2_t = wtile(outwT2, (H, H), "outwT2_t", BF)
            outb2_t = wtile(outb2, (128, H), "outb2_t")
            ln1g_t = wtile(ln1g, (128, H), "ln1g_t")
            ln1b_t = wtile(ln1b, (128, H), "ln1b_t")
            ff1wT_t = wtile(ff1wT, (H, H), "ff1wT_t", BF)
            ff1b_t = wtile(ff1b, (128, H), "ff1b_t")
            ff2wT_t = wtile(ff2wT, (H, H), "ff2wT_t", BF)
            ff2b_t = wtile(ff2b, (128, H), "ff2b_t")
            ln2g_t = wtile(ln2g, (128, H), "ln2g_t")
            ln2b_t = wtile(ln2b, (128, H), "ln2b_t")

            ag3_in = dp.tile([GB, H, NROI], BF, name="ag3_in")
            ag3_out = dp.tile([B, H, NROI], BF, name="ag3_out",
                              addr_space="Shared")
            CH = ((0, 128), (128, SB2))

            def transpose_pair(dst, srcA, srcB, nm):
                tpA = psp.tile([128, 128], FP, name=f"tpA_{nm}", tag="tr",
                               space="PSUM")
                nc.tensor.transpose(tpA[:], srcA, ident[:])
                nc.vector.tensor_copy(dst[:, :128], tpA[:])
                tpB = psp.tile([128, SB2], FP, name=f"tpB_{nm}", tag="tr",
                               space="PSUM")
                nc.tensor.transpose(tpB[:], srcB, ident[:SB2, :SB2])
                nc.vector.tensor_copy(dst[:, 128:], tpB[:])

            def layernorm(xc, sz, gt, bt, nm):
                s = wp.tile([128, 1], FP, name=f"s_{nm}", bufs=2)
                nc.vector.reduce_sum(s[:sz], xc, axis=mybir.AxisListType.X)
                nmu = wp.tile([128, 1], FP, name=f"nmu_{nm}", bufs=2)
                nc.vector.tensor_scalar_mul(nmu[:sz], s[:sz], -1.0 / H)
                sqs = wp.tile([128, H], FP, name=f"sqs_{nm}", bufs=2)
                sq = wp.tile([128, 1], FP, name=f"sq_{nm}", bufs=2)
                nc.scalar.activation(sqs[:sz], xc,
                                     mybir.ActivationFunctionType.Square,
                                     accum_out=sq[:sz])
                var = wp.tile([128, 1], FP, name=f"var_{nm}", bufs=2)
                nc.vector.tensor_scalar_mul(var[:sz], sq[:sz], 1.0 / H)
                mu2 = wp.tile([128, 1], FP, name=f"mu2_{nm}", bufs=2)
                nc.vector.tensor_tensor(out=mu2[:sz], in0=nmu[:sz], in1=nmu[:sz],
                                        op=mybir.AluOpType.mult)
                nc.vector.tensor_tensor(out=var[:sz], in0=var[:sz], in1=mu2[:sz],
                                        op=mybir.AluOpType.subtract)
                rstd = wp.tile([128, 1], FP, name=f"rstd_{nm}", bufs=2)
                nc.scalar.activation(rstd[:sz], var[:sz],
                                     mybir.ActivationFunctionType.Sqrt,
                                     bias=eps_t[:sz, :1])
                nc.vector.reciprocal(rstd[:sz], rstd[:sz])
                nc.vector.tensor_scalar(out=xc, in0=xc, scalar1=nmu[:sz, :1],
                                        scalar2=rstd[:sz, :1],
                                        op0=mybir.AluOpType.add,
                                        op1=mybir.AluOpType.mult)
                nc.vector.tensor_tensor(out=xc, in0=xc, in1=gt[:sz],
                                        op=mybir.AluOpType.mult)
                nc.vector.tensor_tensor(out=xc, in0=xc, in1=bt[:sz],
                                        op=mybir.AluOpType.add)

            for g in range(GB):
                cmbT = wp.tile([128, NROI], BF, name="cmbT", bufs=2)
                transpose_pair(cmbT, cmb[:, g, 0], cmb[:SB2, g, 1], "cmb")
                qT2 = wp.tile([128, NROI], BF, name="qT2", bufs=2)
                kT2 = wp.tile([128, NROI], BF, name="kT2", bufs=2)
                pq2 = psp.tile([128, NROI], FP, name="pq2", tag="acc2",
                               space="PSUM")
                nc.tensor.matmul(pq2[:], lhsT=WqT2_t[:], rhs=cmbT[:],
                                 start=True, stop=True)
                nc.scalar.activation(qT2[:], pq2[:],
                                     mybir.ActivationFunctionType.Identity,
                                     bias=bq2_t[:, :1])
                pk2 = psp.tile([128, NROI], FP, name="pk2", tag="acc2",
                               space="PSUM")
                nc.tensor.matmul(pk2[:], lhsT=WkT2_t[:], rhs=cmbT[:],
                                 start=True, stop=True)
                nc.scalar.activation(kT2[:], pk2[:],
                                     mybir.ActivationFunctionType.Identity,
                                     bias=bk2_t[:, :1])
                vv = wp.tile([128, 2, H], BF, name="vv", bufs=2)
                for ci, (off, sz) in enumerate(CH):
                    pv2 = psp.tile([128, H], FP, name="pv2", tag="acc2",
                                   space="PSUM")
                    nc.tensor.matmul(pv2[:sz], lhsT=cmbT[:, off:off + sz],
                                     rhs=WvT2_t[:], start=True, stop=True)
                    nc.vector.tensor_tensor(out=vv[:sz, ci], in0=pv2[:sz],
                                            in1=bv2_t[:sz],
                                            op=mybir.AluOpType.add)
                oo = wp.tile([128, 2, H], FP, name="oo", bufs=2)
                for h in range(NHEADS):
                    hs = slice(h * HD, (h + 1) * HD)
                    wA = wp.tile([128, NROI], FP, name="wA", bufs=2)
                    wB = wp.tile([SB2, NROI], FP, name="wB", bufs=2)
                    for (off, sz), wt_ in zip(CH, (wA, wB)):
                        ps2 = psp.tile([128, NROI], FP, name="ps2", tag="acc2",
                                       space="PSUM")
                        nc.tensor.matmul(ps2[:sz], lhsT=qT2[hs, off:off + sz],
                                         rhs=kT2[hs, :], start=True, stop=True,
                                         tile_position=(h * HD, 0))
                        rs2 = wp.tile([128, 1], FP, name="rs2", bufs=2)
                        nc.scalar.activation(wt_[:sz], ps2[:sz],
                                             mybir.ActivationFunctionType.Exp,
                                             bias=0.0, scale=ISCALE,
                                             accum_out=rs2[:sz])
                        nc.vector.reciprocal(rs2[:sz], rs2[:sz])
                        nc.vector.tensor_scalar_mul(wt_[:sz], wt_[:sz],
                                                    rs2[:sz, :1])
                        nc.sync.dma_start(o_attnw[g, h, off:off + sz], wt_[:sz])
                    aT0 = wp.tile([128, NROI], BF, name="aT0", bufs=2)
                    transpose_pair(aT0, wA[:, :128], wB[:, :128], "a0")
                    aT1 = wp.tile([SB2, NROI], BF, name="aT1", bufs=2)
                    tpC = psp.tile([SB2, 128], FP, name="tpC", tag="tr",
                                   space="PSUM")
                    nc.tensor.matmul(tpC[:], lhsT=wA[:, 128:], rhs=ident[:],
                                     is_transpose=True, start=True, stop=True)
                    nc.vector.tensor_copy(aT1[:, :128], tpC[:])
                    tpD = psp.tile([SB2, SB2], FP, name="tpD", tag="tr",
                                   space="PSUM")
                    nc.tensor.matmul(tpD[:], lhsT=wB[:, 128:],
                                     rhs=ident[:SB2, :SB2],
                                     is_transpose=True, start=True, stop=True)
                    nc.vector.tensor_copy(aT1[:, 128:], tpD[:])
                    for ci, (off, sz) in enumerate(CH):
                        po2 = psp.tile([128, HD], FP, name="po2", tag="acc2",
                                       space="PSUM")
                        nc.tensor.matmul(po2[:sz], lhsT=aT0[:, off:off + sz],
                                         rhs=vv[:, 0, hs], start=True,
                                         stop=False)
                        nc.tensor.matmul(po2[:sz], lhsT=aT1[:, off:off + sz],
                                         rhs=vv[:SB2, 1, hs], start=False,
                                         stop=True)
                        nc.vector.tensor_copy(oo[:sz, ci, hs], po2[:sz])
                ooT = wp.tile([128, NROI], BF, name="ooT", bufs=2)
                transpose_pair(ooT, oo[:, 0], oo[:SB2, 1], "oo")
                x1 = wp.tile([128, 2, H], FP, name="x1", bufs=2)
                for ci, (off, sz) in enumerate(CH):
                    pa2 = psp.tile([128, H], FP, name="pa2", tag="acc2",
                                   space="PSUM")
                    nc.tensor.matmul(pa2[:sz], lhsT=ooT[:, off:off + sz],
                                     rhs=outwT2_t[:], start=True, stop=True)
                    nc.vector.tensor_tensor(out=x1[:sz, ci], in0=pa2[:sz],
                                            in1=outb2_t[:sz],
                                            op=mybir.AluOpType.add)
                    nc.vector.tensor_tensor(out=x1[:sz, ci], in0=x1[:sz, ci],
                                            in1=cmb[:sz, g, ci],
                                            op=mybir.AluOpType.add)
                    layernorm(x1[:sz, ci], sz, ln1g_t, ln1b_t, "ln1")
                x1T = wp.tile([128, NROI], BF, name="x1T", bufs=2)
                transpose_pair(x1T, x1[:, 0], x1[:SB2, 1], "x1")
                f1 = wp.tile([128, 2, H], FP, name="f1", bufs=2)
                for ci, (off, sz) in enumerate(CH):
                    pf2 = psp.tile([128, H], FP, name="pf2", tag="acc2",
                                   space="PSUM")
                    nc.tensor.matmul(pf2[:sz], lhsT=x1T[:, off:off + sz],
                                     rhs=ff1wT_t[:], start=True, stop=True)
                    nc.vector.tensor_tensor(out=f1[:sz, ci], in0=pf2[:sz],
                                            in1=ff1b_t[:sz],
                                            op=mybir.AluOpType.add)
                    nc.scalar.activation(f1[:sz, ci], f1[:sz, ci],
                                         mybir.ActivationFunctionType.Relu)
                f1T = wp.tile([128, NROI], BF, name="f1T", bufs=2)
                transpose_pair(f1T, f1[:, 0], f1[:SB2, 1], "f1")
                x3 = wp.tile([128, 2, H], FP, name="x3", bufs=2)
                for ci, (off, sz) in enumerate(CH):
                    pf22 = psp.tile([128, H], FP, name="pf22", tag="acc2",
                                    space="PSUM")
                    nc.tensor.matmul(pf22[:sz], lhsT=f1T[:, off:off + sz],
                                     rhs=ff2wT_t[:], start=True, stop=True)
                    nc.vector.tensor_tensor(out=x3[:sz, ci], in0=pf22[:sz],
                                            in1=ff2b_t[:sz],
                                            op=mybir.AluOpType.add)
                    nc.vector.tensor_tensor(out=x3[:sz, ci], in0=x3[:sz, ci],
                                            in1=x1[:sz, ci],
                                            op=mybir.AluOpType.add)
                    layernorm(x3[:sz, ci], sz, ln2g_t, ln2b_t, "ln2")
                    nc.sync.dma_start(o_tout[g, off:off + sz], x3[:sz, ci])
                toutT = wp.tile([128, NROI], BF, name="toutT", bufs=2)
                transpose_pair(toutT, x3[:, 0], x3[:SB2, 1], "to")
                nc.sync.dma_start(ag3_in[g], toutT[:])

            nc.gpsimd.collective_compute(
                "AllGather", mybir.AluOpType.bypass, replica_groups=RG,
                ins=[ag3_in.opt()], outs=[ag3_out.opt()])

            # ---------------- MLP + AllReduce
            rhs_all = pp.tile([128, B, NROI], BF, name="rhs_all")
            nc.sync.dma_start(rhs_all[:], ag3_out.rearrange("g f r -> f g r"))
            msc_t = wtile(mlp_scale, (128, 1), "msc_t")
            mbi_t = wtile(mlp_bias, (128, 1), "mbi_t")
            w2sT_t = wtile(w2sT, (128, OUT_D), "w2sT_t")
            phc = psp.tile([128, B], FP, name="phc", tag="acc", space="PSUM")
            NCHUNK = 8
            for j in range((NROI + NCHUNK - 1) // NCHUNK):
                r0 = j * NCHUNK
                nch = min(NCHUNK, NROI - r0)
                w1c = wp.tile([128, NCHUNK * 128], BF, name="w1c")
                nc.sync.dma_start(
                    w1c[:, :nch * 128].rearrange("p (c m) -> p c m", c=nch),
                    w1sT[r0 * 128:(r0 + nch) * 128].rearrange(
                        "(c p) m -> p c m", p=128))
                for cth in range(nch):
                    r = r0 + cth
                    nc.tensor.matmul(
                        phc[:], lhsT=w1c[:, cth * 128:(cth + 1) * 128],
                        rhs=rhs_all[:, :, r], start=(r == 0),
                        stop=(r == NROI - 1))
            zt = wp.tile([128, B], FP, name="zt", bufs=1)
            nc.scalar.activation(zt[:], phc[:],
                                 mybir.ActivationFunctionType.Identity,
                                 bias=mbi_t[:, :1], scale=msc_t[:, :1])
            hc_p = wp.tile([128, B], FP, name="hc_p", bufs=1)
            nc.vector.tensor_scalar_max(hc_p[:], zt[:], 0.0)
            hc_n = wp.tile([128, B], FP, name="hc_n", bufs=1)
            nc.vector.tensor_scalar(out=hc_n[:], in0=zt[:], scalar1=0.0,
                                    scalar2=0.01, op0=mybir.AluOpType.min,
                                    op1=mybir.AluOpType.mult)
            nc.vector.tensor_tensor(out=hc_p[:], in0=hc_p[:], in1=hc_n[:],
                                    op=mybir.AluOpType.add)
            plog = psp.tile([B, OUT_D], FP, name="plog", tag="acc2",
                            space="PSUM")
            nc.tensor.matmul(plog[:], lhsT=hc_p[:], rhs=w2sT_t[:],
                             start=True, stop=True)
            log_sb = wp.tile([B, OUT_D], FP, name="log_sb", bufs=1)
            nc.vector.tensor_copy(log_sb[:], plog[:])
            ar_in = dp.tile([B, OUT_D], FP, name="ar_in")
            ar_out = dp.tile([B, OUT_D], FP, name="ar_out", addr_space="Shared")
            nc.sync.dma_start(ar_in[:], log_sb[:])
            nc.gpsimd.collective_compute(
                "AllReduce", mybir.AluOpType.add, replica_groups=RG,
                ins=[ar_in.opt()], outs=[ar_out.opt()])
            logf = wp.tile([B, OUT_D], FP, name="logf", bufs=1)
            nc.sync.dma_start(logf[:], ar_out[:])
            b2o_t = wtile(b2_out, (B, OUT_D), "b2o_t")
            nc.vector.tensor_tensor(out=logf[:], in0=logf[:], in1=b2o_t[:],
                                    op=mybir.AluOpType.add)
            nc.sync.dma_start(o_logits[:], logf[:])

    nc.compile()
    return nc


# ---------------------------------------------------------------- entry point

def kernel(_trace=False, **inputs):
    in_maps, cfg = _prep(inputs)
    if cfg not in _CACHE:
        _CACHE[cfg] = _build(cfg)
    nc = _CACHE[cfg]
    res = bass_utils.run_bass_kernel_spmd(
        nc, in_maps, core_ids=list(range(NCORE)), trace=_trace)
    kernel.last_results = res
    r = res.results
    cat = lambda k: np.concatenate([r[c][k] for c in range(NCORE)], axis=0)
    out = r[0]["o_logits"]
    embedding = cat("o_emb")
    embedding_roi = cat("o_embroi")
    combined = cat("o_comb")
    transformer_out = cat("o_tout")
    attn_w = cat("o_attnw")
    embedding_3h = cat("o_emb3h")
    w3h = cat("o_w3h")
    return (out, embedding, embedding_roi, combined, transformer_out,
            attn_w, embedding_3h, w3h)
